# revision 46
# baseline (speedup 1.0000x reference)
"""Causal multi-head attention (RoPE) on 8 Trainium2 NeuronCores — fp8 edition.

Sharding: (batch=2) x (head groups=4) -> 8 cores; core c = 4*b + g handles
batch b, heads [4g, 4g+4). Each core computes its 4 heads' attention plus its
partial o_proj contribution; the host sums 4 partials per batch.

Per-core kernel (single Tile program, SPMD over cores):
  - QKV projections and the S / AV attention matmuls run in fp8e4m3 with
    MatmulPerfMode.DoubleRow (2 contraction tiles per instruction at 0.5
    cycles/row -> 4x the f32r/bf16 matmul throughput).  Weights are
    pre-scaled by 16 on the host to center them in fp8 range; the scale is
    folded into the exp() activation scale (2^-11) and a 16.0 ones-column
    that yields the softmax denominator.
  - Queries/keys 0-127 run through a separate bf16 "hi-precision" path
    (fp8 noise does not average out over few softmax terms at early rows).
  - Causal masking of diagonal key-blocks is done on the PE: a constant
    fp8 DoubleRow matmul accumulates -115200 (* 2^-11 -> -56) into masked
    score positions, so exp() flushes them to exact zero.  No vector-engine
    mask pass.
  - o_proj stays f32r (accuracy); softmax epilogue = reciprocal (DVE) +
    partition_broadcast (Pool) + multiply (DVE).
  - RoPE: PSUM->SBUF bf16 copy (DVE), 64-partition swap via SBUF-SBUF DMA,
    bf16 table multiplies (DVE 2x mode), two adds -> fp8 (Pool).
  - Emission is software-pipelined: the 32 (chunk, head) attention units
    form one stream with one score-group of lookahead, and projection /
    hi-path / o_proj work is spread across unit boundaries (projections
    front-loaded) so the Activation engine — the critical engine at ~78 us
    of exp() — stays fed.  Hardware quirks found on the way: GPSIMD cannot
    touch PSUM; a PSUM accumulation bracket must keep one tile_position
    (mixing base partitions crashes the device); DoubleRow Ldweights needs
    a >=128-element pair stride.

Layouts (per core):
  qrot/krot: two [64, 2, seq] fp8 tiles (head pair kt); head h on
    partitions 32(h%2)..+32, plane t in {0,1} = rotated-even/odd dims;
    S = DoubleRow over (32 partitions x 2 planes) = the 64-dim contraction.
  vt [128, 8, 4, 2, 128] fp8: key-block pair, head, block-in-pair; cols
    0:64 = v-dims, col 64 = 16.0 ones column (softmax denominator).
  S^T tiles [128 keys, W queries] accumulate 4 key-blocks per PSUM group;
  one exp() per group feeds DoubleRow AV over key-block pairs.
"""

import sys

for _p in ("/opt/trn_rl_repo",):
    if _p not in sys.path:
        sys.path.insert(0, _p)

import numpy as np
import ml_dtypes

F8 = ml_dtypes.float8_e4m3
BF16 = ml_dtypes.bfloat16

SEQ = 2048
D_MODEL = 1024
NUM_HEADS = 16
HEAD_DIM = 64
THETA = 10000.0
N_CORES = 8
HEADS_PER_CORE = 4
KSLICE = HEADS_PER_CORE * HEAD_DIM  # 256 projection rows per core
PC = 512          # projection chunk width (4 chunks)
CH = 256          # attention query-chunk width (8 chunks)
WSCALE = 16.0     # host pre-scale on wq/wk/wv (fp8 range centering)
EXP_SCALE = 0.125 / (WSCALE * WSCALE)  # folds q*k scale^2 and 1/sqrt(hd)
MASKV = 240.0     # fp8e4m3 max-ish; DR mask adds -2*240*240 = -115200


def build_nc(seq=SEQ):
    import os
    BISECT = int(os.environ.get("KBISECT", "3"))  # 1: proj only, 2: +S/exp, 3: full
    KHI = int(os.environ.get("KHI", "1"))
    KHIPART = int(os.environ.get("KHIPART", "4"))
    KPROJ = int(os.environ.get("KPROJ", "1"))
    KOPROJ = int(os.environ.get("KOPROJ", "1"))
    import concourse.mybir as mybir
    import concourse.tile as tile
    from concourse import bacc
    from contextlib import ExitStack

    f32 = mybir.dt.float32
    f32r = mybir.dt.float32r
    bf16 = mybir.dt.bfloat16
    f8 = mybir.dt.float8e4
    DR = mybir.MatmulPerfMode.DoubleRow
    Exp = mybir.ActivationFunctionType.Exp

    npc = seq // PC              # 4 projection chunks
    nch = seq // CH              # 8 attention chunks
    nib = seq // 128             # 16 key/row blocks

    nc = bacc.Bacc(None, target_bir_lowering=False)

    # fp8 path inputs
    xt8 = nc.declare_dram_parameter("xt8", [npc, 128, 8, PC], f8, isOutput=False)
    wq8 = nc.declare_dram_parameter("wq8", [128, 8, KSLICE], f8, isOutput=False)
    wk8 = nc.declare_dram_parameter("wk8", [128, 8, KSLICE], f8, isOutput=False)
    wv8 = nc.declare_dram_parameter("wv8", [128, 8, KSLICE], f8, isOutput=False)
    tab = nc.declare_dram_parameter("tab", [npc, 128, 2, PC], bf16, isOutput=False)
    tmsk = nc.declare_dram_parameter("tmsk", [128, 2, 2, CH], f8, isOutput=False)
    negi = nc.declare_dram_parameter("negi", [128, 2, 128], f8, isOutput=False)
    # o_proj (f32r)
    woT = nc.declare_dram_parameter("woT", [2, 128, D_MODEL], f32r, isOutput=False)
    # hi-precision path (rows/keys 0-127), bf16
    xhi = nc.declare_dram_parameter("xhi", [128, 8, 128], bf16, isOutput=False)
    whq = nc.declare_dram_parameter("whq", [128, 8, KSLICE], bf16, isOutput=False)
    whk = nc.declare_dram_parameter("whk", [128, 8, KSLICE], bf16, isOutput=False)
    whv = nc.declare_dram_parameter("whv", [128, 8, KSLICE], bf16, isOutput=False)
    htab = nc.declare_dram_parameter("htab", [128, 2, 128], bf16, isOutput=False)

    y = nc.declare_dram_parameter("y", [seq, D_MODEL], f32, isOutput=True)

    with tile.TileContext(nc) as tc, ExitStack() as ctx:
        persist = ctx.enter_context(tc.tile_pool(name="persist", bufs=1))

        qrot = [persist.tile([64, 2, seq], f8, tag=f"qrot{k}", name=f"qrot{k}")
                for k in range(2)]
        krot = [persist.tile([64, 2, seq], f8, tag=f"krot{k}", name=f"krot{k}")
                for k in range(2)]
        # AV DoubleRow weights: k-block PAIR on dim3 with 128-wide inner
        # stride (walrus ISA check rejects <128 pair strides); cols 0:64 =
        # v-dims, col 64 = 16.0 ones (softmax denominator), 65:128 unused.
        vt = persist.tile([128, nib // 2, HEADS_PER_CORE, 2, 128], f8, tag="vt")
        outTn = [persist.tile([128, seq], f32r, tag=f"outTnP{p}", name=f"outTnP{p}")
                 for p in range(HEADS_PER_CORE // 2)]
        nc.vector.memset(vt[:, :, :, :, 64:65], WSCALE)

        wq_s = persist.tile([128, 8, KSLICE], f8, tag="wq_s")
        wk_s = persist.tile([128, 8, KSLICE], f8, tag="wk_s")
        wv_s = persist.tile([128, 8, KSLICE], f8, tag="wv_s")
        wo_s = persist.tile([128, 2, D_MODEL], f32r, tag="wo_s")
        tm_s = persist.tile([128, 2, 2, CH], f8, tag="tm_s")
        ni_s = persist.tile([128, 2, 128], f8, tag="ni_s")
        # load order = dependency order: first projection chunk's x and the
        # q/k weights gate everything; wo/o_proj and hi-path gear come later.
        nc.sync.dma_start(out=wq_s[:], in_=wq8[:])
        nc.sync.dma_start(out=wk_s[:], in_=wk8[:])

        # hi-path persistent
        xhi_s = persist.tile([128, 8, 128], bf16, tag="xhi_s")
        whq_s = persist.tile([128, 8, KSLICE], bf16, tag="whq_s")
        whk_s = persist.tile([128, 8, KSLICE], bf16, tag="whk_s")
        whv_s = persist.tile([128, 8, KSLICE], bf16, tag="whv_s")
        htab_s = persist.tile([128, 2, 128], bf16, tag="htab_s")
        qhi = [persist.tile([128, 128], bf16, tag=f"qhi{k}", name=f"qhi{k}") for k in range(2)]
        khi = [persist.tile([128, 128], bf16, tag=f"khi{k}", name=f"khi{k}") for k in range(2)]
        vthi = persist.tile([128, HEADS_PER_CORE, 65], bf16, tag="vthi")
        nc.vector.memset(vthi[:, :, 64:65], 1.0)

        with (
            tc.tile_pool(name="p1x", bufs=2) as p1x,
            tc.tile_pool(name="rtmp", bufs=8) as rtmp,
            tc.tile_pool(name="ptpool", bufs=4) as ptpool,
            tc.tile_pool(name="lpool", bufs=4) as lpool,
            tc.tile_pool(name="ystage", bufs=4) as yst,
            tc.tile_pool(name="sgpsum", bufs=2, space="PSUM") as sgp,
            tc.tile_pool(name="popsum", bufs=2, space="PSUM") as pop,
            tc.tile_pool(name="shpsum", bufs=2, space="PSUM") as shp,
        ):
            if BISECT < 3 or not KHI:
                zt = rtmp.tile([128, 512], f32, tag="zfill")
                nc.vector.memset(zt[:], 0.0)
                for p_ in range(2):
                    for cc in range(4):
                        nc.vector.tensor_copy(
                            out=outTn[p_][:, cc * 512:cc * 512 + 512], in_=zt[:])

            # ---------------- hi-precision path: rows/keys 0-127 ------------
            # Emitted as aux items interleaved with early attention chunks
            # (its long serial chain would otherwise stall the pipe start).
            U = persist.tile([128, 128], f32, tag="umask")
            nc.gpsimd.memset(U[:], 0.0)
            nc.gpsimd.affine_select(
                out=U[:], in_=U[:], compare_op=mybir.AluOpType.is_ge,
                fill=-1e9, base=0, pattern=[[1, 128]], channel_multiplier=-1,
            )
            pthi = ptpool.tile([128, 4, 128], bf16, tag="pthi")

            def emit_hi_qk(kt):
                # projections (bf16, contraction d=1024 over 8 d-tiles)
                for w_s, dst in ((whq_s, qhi[kt]), (whk_s, khi[kt])):
                    pp = shp.tile([128, 512], f32, tag="pp", name=f"hiqk{kt}")
                    for dt in range(8):
                        nc.tensor.matmul(
                            pp[:, 0:128],
                            lhsT=w_s[:, dt, kt * 128:kt * 128 + 128],
                            rhs=xhi_s[:, dt, :],
                            start=(dt == 0), stop=(dt == 7),
                        )
                    # rope (baseline-perm rows: [h0ev h0od h1ev h1od], 32-swap)
                    praw = rtmp.tile([128, 128], bf16, tag="hpraw")
                    nc.vector.tensor_copy(out=praw[:], in_=pp[:, 0:128])
                    swp = rtmp.tile([128, 128], bf16, tag="hswp")
                    for blk in range(4):
                        d, s = blk * 32, blk * 32 + (32 if blk % 2 == 0 else -32)
                        nc.vector.tensor_copy(out=swp[d:d + 32, :], in_=praw[s:s + 32, :])
                    pc_ = rtmp.tile([128, 128], bf16, tag="hpc")
                    ps_ = rtmp.tile([128, 128], bf16, tag="hps")
                    nc.gpsimd.tensor_mul(pc_[:], praw[:], htab_s[:, 0, :])
                    nc.gpsimd.tensor_mul(ps_[:], swp[:], htab_s[:, 1, :])
                    nc.vector.tensor_add(dst[:], pc_[:], ps_[:])

            def emit_hi_v():
                vp = shp.tile([128, KSLICE], f32, tag="pp", name="hiv")
                for dt in range(8):
                    nc.tensor.matmul(
                        vp[:], lhsT=xhi_s[:, dt, :], rhs=whv_s[:, dt, :],
                        start=(dt == 0), stop=(dt == 7),
                    )
                nc.scalar.copy(
                    out=vthi[:, :, 0:64],
                    in_=vp[:].rearrange("p (h z) -> p h z", z=64),
                )

            def emit_hi_s():
                # NB: one accumulation bracket per head — mixing tile_position
                # (base partition) inside a PSUM bracket crashes the hardware.
                for h in range(4):
                    shi = shp.tile([128, 512], f32, tag="pp", name=f"shi{h}")
                    nc.tensor.matmul(
                        shi[:, 0:128],
                        lhsT=khi[h // 2][64 * (h % 2):64 * (h % 2) + 64, :],
                        rhs=qhi[h // 2][64 * (h % 2):64 * (h % 2) + 64, :],
                        start=True, stop=True,
                    )
                    nc.vector.tensor_add(shi[:, 0:128], shi[:, 0:128], U[:])
                    nc.scalar.activation(out=pthi[:, h, :], in_=shi[:, 0:128],
                                         func=Exp, scale=0.125)

            def emit_hi_av():
                pohi = shp.tile([65, 4, 128], f32, tag="pp", name="pohi")
                for h in range(4):
                    nc.tensor.matmul(
                        pohi[:, h, :], lhsT=vthi[:, h, :], rhs=pthi[:, h, :],
                        start=(h == 0), stop=(h == 3),
                    )
                for h in range(4):
                    li = lpool.tile([1, 128], f32, tag="li", name="hli")
                    nc.vector.reciprocal(out=li[:], in_=pohi[64:65, h, :])
                    lb = lpool.tile([64, 128], f32, tag="lb", name="hlb")
                    nc.gpsimd.partition_broadcast(lb[:], li[:])
                    hb = 64 * (h % 2)
                    nc.vector.tensor_mul(
                        outTn[h // 2][hb:hb + 64, 0:128], pohi[0:64, h, :], lb[:]
                    )

            # ---------------- fp8 projections, per 512-chunk ----------------
            xts, tbs = {}, {}

            def emit_loads(p):
                if p >= npc or p in xts:
                    return
                xt = p1x.tile([128, 8, PC], f8, tag="xt")
                nc.sync.dma_start(out=xt[:], in_=xt8[p])
                tb = p1x.tile([128, 2, PC], bf16, tag="tb")
                nc.sync.dma_start(out=tb[:], in_=tab[p])
                xts[p], tbs[p] = xt, tb

            def emit_proj_kt(p, kt):
                """Q and K projection + rope for head-pair kt of chunk p,
                phase-interleaved (both PSUM copies run while the swap DMAs
                fly, so the table-multiplies rarely stall).  High priority:
                this chain gates whole chunks of attention."""
                xt, tb = xts[p], tbs[p]
                csl = slice(p * PC, p * PC + PC)
                pps, praws, swps = [], [], []
                for w_s in (wq_s, wk_s):
                    # one PSUM group across both 256-wide halves (shared 2KB
                    # zero region: start once, stop on the final matmul)
                    pp = shp.tile([128, 512], f32, tag="pp")
                    for hf in range(2):
                        for a in range(4):
                            nc.tensor.matmul(
                                pp[:, hf * 256:hf * 256 + 256],
                                lhsT=w_s[:, 2 * a:2 * a + 2, kt * 128:kt * 128 + 128],
                                rhs=xt[:, 2 * a:2 * a + 2, hf * 256:hf * 256 + 256],
                                start=(hf == 0 and a == 0),
                                stop=(hf == 1 and a == 3),
                                perf_mode=DR, skip_group_check=True,
                            )
                    pps.append(pp)
                for i in range(2):
                    praw = rtmp.tile([128, PC], bf16, tag="praw")
                    nc.vector.tensor_copy(out=praw[:], in_=pps[i][:])
                    praws.append(praw)
                    swp = rtmp.tile([128, PC], bf16, tag="swp")
                    nc.sync.dma_start(out=swp[0:64, :], in_=praw[64:128, :])
                    nc.sync.dma_start(out=swp[64:128, :], in_=praw[0:64, :])
                    swps.append(swp)
                for i, rot in enumerate((qrot, krot)):
                    pc_ = rtmp.tile([128, PC], bf16, tag="pc")
                    ps_ = rtmp.tile([128, PC], bf16, tag="ps")
                    nc.vector.tensor_mul(pc_[:], praws[i][:], tb[:, 0, :])
                    nc.vector.tensor_mul(ps_[:], swps[i][:], tb[:, 1, :])
                    # heads (2kt, 2kt+1) live in tile kt on partitions 0-63;
                    # plane t0=even-rot, t1=odd-rot.
                    # praw rows: [h0ev h1ev | h0od h1od].
                    nc.gpsimd.tensor_add(
                        rot[kt][:, 0, csl], pc_[0:64, :], ps_[0:64, :])
                    nc.gpsimd.tensor_add(
                        rot[kt][:, 1, csl], pc_[64:128, :], ps_[64:128, :])

            def emit_proj_v(p, half):
                """V projection for 2 of chunk p's 4 position-blocks."""
                xt = xts[p]
                for ibl in (2 * half, 2 * half + 1):
                    ib = p * (PC // 128) + ibl
                    vp = shp.tile([128, KSLICE], f32, tag="pp", name="vp")
                    for a in range(4):
                        nc.tensor.matmul(
                            vp[:],
                            lhsT=xt[:, 2 * a:2 * a + 2, ibl * 128:ibl * 128 + 128],
                            rhs=wv_s[:, 2 * a:2 * a + 2, :],
                            start=(a == 0), stop=(a == 3),
                            perf_mode=DR, skip_group_check=True,
                        )
                    nc.scalar.copy(
                        out=vt[:, ib // 2, :, ib % 2, 0:64],
                        in_=vp[:].rearrange("p (h z) -> p h z", z=64),
                    )

            def emit_proj(p):
                emit_proj_kt(p, 0)
                emit_proj_kt(p, 1)
                emit_proj_v(p, 0)
                emit_proj_v(p, 1)

            emit_loads(0)
            emit_loads(1)
            nc.sync.dma_start(out=wv_s[:], in_=wv8[:])
            nc.sync.dma_start(out=tm_s[:], in_=tmsk[:])
            nc.sync.dma_start(out=ni_s[:], in_=negi[:])
            emit_proj(0)

            def emit_hi_loads():
                nc.sync.dma_start(out=xhi_s[:], in_=xhi[:])
                nc.sync.dma_start(out=whq_s[:], in_=whq[:])
                nc.sync.dma_start(out=whk_s[:], in_=whk[:])
                nc.sync.dma_start(out=whv_s[:], in_=whv[:])
                nc.sync.dma_start(out=htab_s[:], in_=htab[:])

            def emit_wo_load():
                nc.sync.dma_start(out=wo_s[:], in_=woT[:].rearrange("q p d -> p q d"))

            # ---------------- attention: pipelined (chunk, head) units -------
            def make_unit(c, h):
                """Returns (ngrp, emit_sg, emit_expav) closures for one
                attention unit: chunk c (q-window), head h."""
                q0 = c * CH + (128 if c == 0 else 0)   # hi-path covers rows 0-127
                W = c * CH + CH - q0
                njb = 2 * (c + 1)                       # causal key blocks
                ngrp = (njb + 3) // 4
                hp = slice(32 * (h % 2), 32 * (h % 2) + 32)
                qr, kr = qrot[h // 2], krot[h // 2]
                st = {"po": None, "sg": {}, "pt": {}}

                def emit_sg(G):
                    gn = min(4, njb - 4 * G)
                    sg = sgp.tile([128, 4, 256], f32, tag="sg")
                    for rj in range(0, gn, 2):      # per 2KB psum region
                        jA, jB = 4 * G + rj, 4 * G + rj + 1
                        mms = [
                            (sg[:, rj + s, 0:W],
                             kr[hp, :, j * 128:j * 128 + 128],
                             qr[hp, :, q0:q0 + W])
                            for s, j in ((0, jA), (1, jB))
                        ]
                        for s, j in ((0, jA), (1, jB)):
                            # diagonal-block causal mask matmul
                            if j >= 2 * c and not (c == 0 and j == 0):
                                which = j - 2 * c   # 0: T0, 1: T1 pattern
                                mms.append(
                                    (sg[:, rj + s, 0:W], ni_s[:],
                                     tm_s[:, :, which, CH - W:CH]))
                        for i, (o, l, r) in enumerate(mms):
                            nc.tensor.matmul(
                                o, lhsT=l, rhs=r,
                                start=(i == 0), stop=(i == len(mms) - 1),
                                perf_mode=DR, skip_group_check=True,
                            )
                    st["sg"][G] = sg

                def emit_expav(G):
                    gn = min(4, njb - 4 * G)
                    pt = ptpool.tile([128, 4, 256], f8, tag="pt")
                    nc.scalar.activation(
                        out=pt[:, 0:gn, 0:W], in_=st["sg"].pop(G)[:, 0:gn, 0:W],
                        func=Exp, scale=EXP_SCALE,
                    )
                    if st["po"] is None:
                        st["po"] = pop.tile([65, 512], f32, tag="po", name="po")
                    po = st["po"]
                    for u in range(0, gn, 2):
                        jb = 4 * G + u
                        nc.tensor.matmul(
                            po[:, 0:W],
                            lhsT=vt[:, jb // 2, h, :, 0:65],
                            rhs=pt[:, u:u + 2, 0:W],
                            start=(jb == 0), stop=(jb + 2 >= njb),
                            perf_mode=DR, skip_group_check=True,
                        )
                    if G == ngrp - 1:
                        # epilogue: normalize by the 16.0-ones denominator row
                        li = lpool.tile([1, 256], f32, tag="li")
                        nc.vector.reciprocal(out=li[:, 0:W], in_=po[64:65, 0:W])
                        lb = lpool.tile([64, 256], f32, tag="lb")
                        nc.gpsimd.partition_broadcast(lb[:, 0:W], li[:, 0:W])
                        hb = 64 * (h % 2)
                        nc.vector.tensor_mul(
                            outTn[h // 2][hb:hb + 64, q0:q0 + W], po[0:64, 0:W],
                            lb[:, 0:W],
                        )

                return ngrp, emit_sg, emit_expav

            def emit_oproj(c, ib):
                ys = yst.tile([128, D_MODEL], f32, tag="ys")
                for ns in range(2):
                    yp = shp.tile([128, 512], f32, tag="pp", name="yp")
                    for pr in range(2):
                        nc.tensor.matmul(
                            yp[:],
                            lhsT=outTn[pr][:, ib * 128:ib * 128 + 128],
                            rhs=wo_s[:, pr, ns * 512:ns * 512 + 512],
                            start=(pr == 0), stop=(pr == 1),
                        )
                    nc.vector.tensor_copy(
                        out=ys[:, ns * 512:ns * 512 + 512], in_=yp[:])
                nc.sync.dma_start(
                    out=y[ib * 128:ib * 128 + 128, :], in_=ys[:])

            # Aux PE work is interleaved at unit boundaries.  Projections are
            # front-loaded (the Activation engine idles until later chunks'
            # scores exist, so finishing all projections early flattens the
            # causal-triangular exp schedule); o_proj items fill afterwards.
            from collections import deque

            projq = deque()                         # (proj_idx, closure)
            for p in (1, 2, 3):
                if p >= 2:
                    projq.append((p, lambda p=p: emit_loads(p)))
                projq.append((p, lambda p=p: emit_proj_kt(p, 0)))
                projq.append((p, lambda p=p: emit_proj_kt(p, 1)))
                projq.append((p, lambda p=p: emit_proj_v(p, 0)))
                projq.append((p, lambda p=p: emit_proj_v(p, 1)))
                if p == 1:
                    for fn in (emit_hi_loads, lambda: emit_hi_qk(0),
                               lambda: emit_hi_qk(1), emit_hi_v, emit_hi_s,
                               emit_hi_av, emit_wo_load):
                        projq.append((1, fn))
            oprojq = deque()                        # ready o_proj items

            def drain_proj(upto):
                while projq and projq[0][0] <= upto:
                    projq.popleft()[1]()

            pending = deque()                       # (emit_expav, G)
            for c in range(nch):
                drain_proj(c // 2)                  # hard dependency
                for h in range(HEADS_PER_CORE):
                    ngrp, emit_sg, emit_expav = make_unit(c, h)
                    for G in range(ngrp):
                        emit_sg(G)
                        pending.append((emit_expav, G))
                        while len(pending) > 1:
                            f, g = pending.popleft()
                            f(g)
                        # boundary aux: prefer projections, two per slot.
                        # none during chunk 0 — early aux wedges the in-order
                        # PE queue behind not-yet-loaded x chunks.
                        for _ in range(2):
                            if projq:
                                projq.popleft()[1]()
                            elif oprojq:
                                oprojq.popleft()()
                if c < nch - 1:
                    oprojq.append(lambda c=c: emit_oproj(c, 2 * c))
                    oprojq.append(lambda c=c: emit_oproj(c, 2 * c + 1))
            while pending:
                f, g = pending.popleft()
                f(g)
            while oprojq:
                oprojq.popleft()()
            emit_oproj(nch - 1, 2 * nch - 2)
            emit_oproj(nch - 1, 2 * nch - 1)

    nc.finalize()
    return nc


def _rope_tables(seq, width, swapped_sign_rows):
    """cos/sin tables in [128, seq] row layout; freq index = row mod 32.
    swapped_sign_rows: '64' -> rows 0-63 get -sin (64-swap layout);
    '32' -> rows [32:64] and [96:128] get -sin (32-swap layout)."""
    half = HEAD_DIM // 2
    inv = 1.0 / (THETA ** (2.0 * np.arange(half) / HEAD_DIM))
    ang = np.arange(seq, dtype=np.float64)[:, None] * inv[None, :]  # [seq, 32]
    cos32 = np.cos(ang).T  # [32, seq]
    sin32 = np.sin(ang).T
    cosI = np.tile(cos32, (4, 1))
    if swapped_sign_rows == "64":
        sinI = np.concatenate([-np.tile(sin32, (2, 1)), np.tile(sin32, (2, 1))], 0)
    else:
        # 32-swap layout: swp rows [od, ev, od', ev'] -> signs [-,+,-,+]
        sinI = np.concatenate([-sin32, sin32, -sin32, sin32], 0)
    return cosI[:, :width], sinI[:, :width]


def make_in_maps(in_features, q_proj, k_proj, v_proj, o_proj, token_positions,
                 seq=SEQ):
    x = np.asarray(in_features, np.float32)
    wq = np.asarray(q_proj, np.float32)
    wk = np.asarray(k_proj, np.float32)
    wv = np.asarray(v_proj, np.float32)
    wo = np.asarray(o_proj, np.float32)

    # fp8-path q/k row perm per 128-row ktile: [h0ev(32) h1ev(32) h0od(32) h1od(32)]
    ev = np.arange(0, HEAD_DIM, 2)
    od = np.arange(1, HEAD_DIM, 2)
    perm8 = []
    for kt in range(2):
        h0, h1 = 2 * kt, 2 * kt + 1
        perm8 += [h0 * HEAD_DIM + ev, h1 * HEAD_DIM + ev,
                  h0 * HEAD_DIM + od, h1 * HEAD_DIM + od]
    perm8 = np.concatenate(perm8)  # local perm within a core's 256 rows
    # hi-path perm: [h0ev h0od h1ev h1od]
    permh = []
    for kt in range(2):
        h0, h1 = 2 * kt, 2 * kt + 1
        permh += [h0 * HEAD_DIM + ev, h0 * HEAD_DIM + od,
                  h1 * HEAD_DIM + ev, h1 * HEAD_DIM + od]
    permh = np.concatenate(permh)

    npc = seq // PC
    # x chunked fp8: [npc, 128, 8, PC]
    xt8b, xhib = [], []
    for b in range(x.shape[0]):
        xT = np.ascontiguousarray(x[b].T)                       # [1024, seq]
        xt8b.append(np.ascontiguousarray(
            xT.reshape(8, 128, npc, PC).transpose(2, 1, 0, 3)).astype(F8))
        xhib.append(np.ascontiguousarray(
            xT[:, 0:128].reshape(8, 128, 128).transpose(1, 0, 2)).astype(BF16))

    # rope tables, fp8 path (64-swap): [npc, 128, 2, PC]
    cosI, sinI = _rope_tables(seq, seq, "64")
    tabf = np.stack([cosI, sinI], axis=1)                       # [128, 2, seq]
    tabf = np.ascontiguousarray(
        tabf.reshape(128, 2, npc, PC).transpose(2, 0, 1, 3)).astype(BF16)
    # hi tables (32-swap), width 128
    cosH, sinH = _rope_tables(seq, 128, "32")
    htabf = np.ascontiguousarray(np.stack([cosH, sinH], 1)).astype(BF16)

    # causal mask patterns T0/T1 [128, CH], value MASKV
    k_ = np.arange(128)[:, None]
    j_ = np.arange(CH)[None, :]
    T0 = (j_ < k_).astype(np.float32) * MASKV
    T1 = (j_ < k_ + 128).astype(np.float32) * MASKV
    tmskf = np.ascontiguousarray(
        np.broadcast_to(np.stack([T0, T1], 0)[None], (2, 2, 128, CH))
        .transpose(2, 0, 1, 3)).astype(F8)
    negif = np.ascontiguousarray(
        np.broadcast_to((-MASKV * np.eye(128, dtype=np.float32))[:, None, :],
                        (128, 2, 128))).astype(F8)

    def wtile(w, perm, scale, dtype):
        # rows ks (already core-sliced, perm applied), transposed -> [128, 8, 256]
        wT = np.ascontiguousarray(w[perm].T)                    # [1024, 256]
        return np.ascontiguousarray(
            (wT * scale).reshape(8, 128, KSLICE).transpose(1, 0, 2)).astype(dtype)

    in_maps = []
    for core in range(N_CORES):
        b, g = divmod(core, HEADS_PER_CORE)
        ks = np.arange(g * KSLICE, (g + 1) * KSLICE)
        wq_c, wk_c, wv_c = wq[ks], wk[ks], wv[ks]
        in_maps.append({
            "xt8": xt8b[b],
            "wq8": wtile(wq_c, perm8, WSCALE, F8),
            "wk8": wtile(wk_c, perm8, WSCALE, F8),
            "wv8": wtile(wv_c, np.arange(KSLICE), WSCALE, F8),
            "tab": tabf,
            "tmsk": tmskf,
            "negi": negif,
            "woT": np.ascontiguousarray(
                wo[:, ks].T.reshape(2, 128, D_MODEL)).astype(np.float32),
            "xhi": xhib[b],
            "whq": wtile(wq_c, permh, 1.0, BF16),
            "whk": wtile(wk_c, permh, 1.0, BF16),
            "whv": wtile(wv_c, np.arange(KSLICE), 1.0, BF16),
            "htab": htabf,
        })
    return in_maps


def assemble(results, batch=2):
    ys = []
    for b in range(batch):
        parts = [results[b * HEADS_PER_CORE + g]["y"].astype(np.float64)
                 for g in range(HEADS_PER_CORE)]
        ys.append(np.sum(parts, axis=0, dtype=np.float64).astype(np.float32))
    return np.stack(ys, axis=0)


_NC_CACHE = {}


def get_nc(seq=SEQ):
    import os
    key = (seq, os.environ.get("KBISECT", "3"))
    if key not in _NC_CACHE:
        _NC_CACHE[key] = build_nc(seq)
    return _NC_CACHE[key]


def kernel(**inputs):
    from concourse.bass_utils import run_bass_kernel_spmd

    nc = get_nc()
    in_maps = make_in_maps(**inputs)
    res = run_bass_kernel_spmd(nc, in_maps, list(range(N_CORES)))
    return assemble(res.results)


if __name__ == "__main__":
    rng = np.random.default_rng(0)
    ins = {
        "in_features": rng.standard_normal((2, SEQ, D_MODEL), np.float32),
        "q_proj": (rng.standard_normal((D_MODEL, D_MODEL)) * 0.02).astype(np.float32),
        "k_proj": (rng.standard_normal((D_MODEL, D_MODEL)) * 0.02).astype(np.float32),
        "v_proj": (rng.standard_normal((D_MODEL, D_MODEL)) * 0.02).astype(np.float32),
        "o_proj": (rng.standard_normal((D_MODEL, D_MODEL)) * 0.02).astype(np.float32),
        "token_positions": np.arange(SEQ, dtype=np.int32),
    }
    out = kernel(**ins)
    print("kernel output:", out.shape, out.dtype)


# revision 53
# speedup vs baseline: 1.0186x; 1.0186x over previous
"""Causal multi-head attention (RoPE) on 8 Trainium2 NeuronCores — fp8 edition.

Sharding: (batch=2) x (head groups=4) -> 8 cores; core c = 4*b + g handles
batch b, heads [4g, 4g+4). Each core computes its 4 heads' attention plus its
partial o_proj contribution; the host sums 4 partials per batch.

Per-core kernel (single Tile program, SPMD over cores):
  - QKV projections and the S / AV attention matmuls run in fp8e4m3 with
    MatmulPerfMode.DoubleRow (2 contraction tiles per instruction at 0.5
    cycles/row -> 4x the f32r/bf16 matmul throughput).  Weights are
    pre-scaled by 16 on the host to center them in fp8 range; the scale is
    folded into the exp() activation scale (2^-11) and a 16.0 ones-column
    that yields the softmax denominator.
  - Queries/keys 0-127 run through a separate bf16 "hi-precision" path
    (fp8 noise does not average out over few softmax terms at early rows).
  - Causal masking of diagonal key-blocks is done on the PE: a constant
    fp8 DoubleRow matmul accumulates -115200 (* 2^-11 -> -56) into masked
    score positions, so exp() flushes them to exact zero.  No vector-engine
    mask pass.
  - o_proj stays f32r (accuracy); softmax epilogue = reciprocal (DVE) +
    partition_broadcast (Pool) + multiply (DVE).
  - RoPE: PSUM->SBUF bf16 copy (DVE), 64-partition swap via SBUF-SBUF DMA,
    bf16 table multiplies (DVE 2x mode), two adds -> fp8 (Pool).
  - Emission is software-pipelined: the 32 (chunk, head) attention units
    form one stream with one score-group of lookahead, and projection /
    hi-path / o_proj work is spread across unit boundaries (projections
    front-loaded) so the Activation engine — the critical engine at ~78 us
    of exp() — stays fed.  Hardware quirks found on the way: GPSIMD cannot
    touch PSUM; a PSUM accumulation bracket must keep one tile_position
    (mixing base partitions crashes the device); DoubleRow Ldweights needs
    a >=128-element pair stride.

Layouts (per core):
  qrot/krot: two [64, 2, seq] fp8 tiles (head pair kt); head h on
    partitions 32(h%2)..+32, plane t in {0,1} = rotated-even/odd dims;
    S = DoubleRow over (32 partitions x 2 planes) = the 64-dim contraction.
  vt [128, 8, 4, 2, 128] fp8: key-block pair, head, block-in-pair; cols
    0:64 = v-dims, col 64 = 16.0 ones column (softmax denominator).
  S^T tiles [128 keys, W queries] accumulate 4 key-blocks per PSUM group;
  one exp() per group feeds DoubleRow AV over key-block pairs.
"""

import sys

for _p in ("/opt/trn_rl_repo",):
    if _p not in sys.path:
        sys.path.insert(0, _p)

import numpy as np
import ml_dtypes

F8 = ml_dtypes.float8_e4m3
BF16 = ml_dtypes.bfloat16

SEQ = 2048
D_MODEL = 1024
NUM_HEADS = 16
HEAD_DIM = 64
THETA = 10000.0
N_CORES = 8
HEADS_PER_CORE = 4
KSLICE = HEADS_PER_CORE * HEAD_DIM  # 256 projection rows per core
PC = 512          # projection chunk width (4 chunks)
CH = 256          # attention query-chunk width (8 chunks)
WSCALE = 16.0     # host pre-scale on wq/wk/wv (fp8 range centering)
EXP_SCALE = 0.125 / (WSCALE * WSCALE)  # folds q*k scale^2 and 1/sqrt(hd)
MASKV = 240.0     # fp8e4m3 max-ish; DR mask adds -2*240*240 = -115200


def build_nc(seq=SEQ):
    import os
    BISECT = int(os.environ.get("KBISECT", "3"))  # 1: proj only, 2: +S/exp, 3: full
    KHI = int(os.environ.get("KHI", "1"))
    KHIPART = int(os.environ.get("KHIPART", "4"))
    KPROJ = int(os.environ.get("KPROJ", "1"))
    KOPROJ = int(os.environ.get("KOPROJ", "1"))
    import concourse.mybir as mybir
    import concourse.tile as tile
    from concourse import bacc
    from contextlib import ExitStack

    f32 = mybir.dt.float32
    f32r = mybir.dt.float32r
    bf16 = mybir.dt.bfloat16
    f8 = mybir.dt.float8e4
    DR = mybir.MatmulPerfMode.DoubleRow
    Exp = mybir.ActivationFunctionType.Exp

    npc = seq // PC              # 4 projection chunks
    nch = seq // CH              # 8 attention chunks
    nib = seq // 128             # 16 key/row blocks

    nc = bacc.Bacc(None, target_bir_lowering=False)

    # fp8 path inputs
    xt8 = nc.declare_dram_parameter("xt8", [npc, 128, 8, PC], f8, isOutput=False)
    wq8 = nc.declare_dram_parameter("wq8", [128, 8, KSLICE], f8, isOutput=False)
    wk8 = nc.declare_dram_parameter("wk8", [128, 8, KSLICE], f8, isOutput=False)
    wv8 = nc.declare_dram_parameter("wv8", [128, 8, KSLICE], f8, isOutput=False)
    tab = nc.declare_dram_parameter("tab", [npc, 128, 2, PC], bf16, isOutput=False)
    tmsk = nc.declare_dram_parameter("tmsk", [128, 2, 2, CH], f8, isOutput=False)
    negi = nc.declare_dram_parameter("negi", [128, 2, 128], f8, isOutput=False)
    # o_proj (f32r)
    woT = nc.declare_dram_parameter("woT", [2, 128, D_MODEL], f32r, isOutput=False)
    # hi-precision path (rows/keys 0-127), bf16
    xhi = nc.declare_dram_parameter("xhi", [128, 8, 128], bf16, isOutput=False)
    whq = nc.declare_dram_parameter("whq", [128, 8, KSLICE], bf16, isOutput=False)
    whk = nc.declare_dram_parameter("whk", [128, 8, KSLICE], bf16, isOutput=False)
    whv = nc.declare_dram_parameter("whv", [128, 8, KSLICE], bf16, isOutput=False)
    htab = nc.declare_dram_parameter("htab", [128, 2, 128], bf16, isOutput=False)

    y = nc.declare_dram_parameter("y", [seq, D_MODEL], f32, isOutput=True)

    with tile.TileContext(nc) as tc, ExitStack() as ctx:
        persist = ctx.enter_context(tc.tile_pool(name="persist", bufs=1))

        qrot = [persist.tile([64, 2, seq], f8, tag=f"qrot{k}", name=f"qrot{k}")
                for k in range(2)]
        krot = [persist.tile([64, 2, seq], f8, tag=f"krot{k}", name=f"krot{k}")
                for k in range(2)]
        # AV DoubleRow weights: k-block PAIR on dim3 with 128-wide inner
        # stride (walrus ISA check rejects <128 pair strides); cols 0:64 =
        # v-dims, col 64 = 16.0 ones (softmax denominator), 65:128 unused.
        vt = persist.tile([128, nib // 2, HEADS_PER_CORE, 2, 128], f8, tag="vt")
        outTn = [persist.tile([128, seq], f32r, tag=f"outTnP{p}", name=f"outTnP{p}")
                 for p in range(HEADS_PER_CORE // 2)]
        nc.vector.memset(vt[:, :, :, :, 64:65], WSCALE)

        wq_s = persist.tile([128, 8, KSLICE], f8, tag="wq_s")
        wk_s = persist.tile([128, 8, KSLICE], f8, tag="wk_s")
        wv_s = persist.tile([128, 8, KSLICE], f8, tag="wv_s")
        wo_s = persist.tile([128, 2, D_MODEL], f32r, tag="wo_s")
        tm_s = persist.tile([128, 2, 2, CH], f8, tag="tm_s")
        ni_s = persist.tile([128, 2, 128], f8, tag="ni_s")
        # load order = dependency order: first projection chunk's x and the
        # q/k weights gate everything; wo/o_proj and hi-path gear come later.
        nc.sync.dma_start(out=wq_s[:], in_=wq8[:])
        nc.sync.dma_start(out=wk_s[:], in_=wk8[:])

        # hi-path persistent
        xhi_s = persist.tile([128, 8, 128], bf16, tag="xhi_s")
        whq_s = persist.tile([128, 8, KSLICE], bf16, tag="whq_s")
        whk_s = persist.tile([128, 8, KSLICE], bf16, tag="whk_s")
        whv_s = persist.tile([128, 8, KSLICE], bf16, tag="whv_s")
        htab_s = persist.tile([128, 2, 128], bf16, tag="htab_s")
        qhi = [persist.tile([128, 128], bf16, tag=f"qhi{k}", name=f"qhi{k}") for k in range(2)]
        khi = [persist.tile([128, 128], bf16, tag=f"khi{k}", name=f"khi{k}") for k in range(2)]
        vthi = persist.tile([128, HEADS_PER_CORE, 65], bf16, tag="vthi")
        nc.vector.memset(vthi[:, :, 64:65], 1.0)

        with (
            tc.tile_pool(name="p1x", bufs=2) as p1x,
            tc.tile_pool(name="rtmp", bufs=8) as rtmp,
            tc.tile_pool(name="ptpool", bufs=4) as ptpool,
            tc.tile_pool(name="lpool", bufs=6) as lpool,
            tc.tile_pool(name="ystage", bufs=4) as yst,
            tc.tile_pool(name="sgpsum", bufs=2, space="PSUM") as sgp,
            tc.tile_pool(name="popsum", bufs=2, space="PSUM") as pop,
            tc.tile_pool(name="shpsum", bufs=2, space="PSUM") as shp,
        ):
            if BISECT < 3 or not KHI:
                zt = rtmp.tile([128, 512], f32, tag="zfill")
                nc.vector.memset(zt[:], 0.0)
                for p_ in range(2):
                    for cc in range(4):
                        nc.vector.tensor_copy(
                            out=outTn[p_][:, cc * 512:cc * 512 + 512], in_=zt[:])

            # ---------------- hi-precision path: rows/keys 0-127 ------------
            # Emitted as aux items interleaved with early attention chunks
            # (its long serial chain would otherwise stall the pipe start).
            U = persist.tile([128, 128], f32, tag="umask")
            nc.gpsimd.memset(U[:], 0.0)
            nc.gpsimd.affine_select(
                out=U[:], in_=U[:], compare_op=mybir.AluOpType.is_ge,
                fill=-1e9, base=0, pattern=[[1, 128]], channel_multiplier=-1,
            )
            pthi = ptpool.tile([128, 4, 128], bf16, tag="pthi")

            def emit_hi_qk(kt):
                # projections (bf16, contraction d=1024 over 8 d-tiles)
                for w_s, dst in ((whq_s, qhi[kt]), (whk_s, khi[kt])):
                    pp = shp.tile([128, 512], f32, tag="pp", name=f"hiqk{kt}")
                    for dt in range(8):
                        nc.tensor.matmul(
                            pp[:, 0:128],
                            lhsT=w_s[:, dt, kt * 128:kt * 128 + 128],
                            rhs=xhi_s[:, dt, :],
                            start=(dt == 0), stop=(dt == 7),
                        )
                    # rope (baseline-perm rows: [h0ev h0od h1ev h1od], 32-swap)
                    praw = rtmp.tile([128, 128], bf16, tag="hpraw")
                    nc.vector.tensor_copy(out=praw[:], in_=pp[:, 0:128])
                    swp = rtmp.tile([128, 128], bf16, tag="hswp")
                    for blk in range(4):
                        d, s = blk * 32, blk * 32 + (32 if blk % 2 == 0 else -32)
                        nc.vector.tensor_copy(out=swp[d:d + 32, :], in_=praw[s:s + 32, :])
                    pc_ = rtmp.tile([128, 128], bf16, tag="hpc")
                    ps_ = rtmp.tile([128, 128], bf16, tag="hps")
                    nc.gpsimd.tensor_mul(pc_[:], praw[:], htab_s[:, 0, :])
                    nc.gpsimd.tensor_mul(ps_[:], swp[:], htab_s[:, 1, :])
                    nc.vector.tensor_add(dst[:], pc_[:], ps_[:])

            def emit_hi_v():
                vp = shp.tile([128, KSLICE], f32, tag="pp", name="hiv")
                for dt in range(8):
                    nc.tensor.matmul(
                        vp[:], lhsT=xhi_s[:, dt, :], rhs=whv_s[:, dt, :],
                        start=(dt == 0), stop=(dt == 7),
                    )
                nc.scalar.copy(
                    out=vthi[:, :, 0:64],
                    in_=vp[:].rearrange("p (h z) -> p h z", z=64),
                )

            def emit_hi_s():
                # NB: one accumulation bracket per head — mixing tile_position
                # (base partition) inside a PSUM bracket crashes the hardware.
                for h in range(4):
                    shi = shp.tile([128, 512], f32, tag="pp", name=f"shi{h}")
                    nc.tensor.matmul(
                        shi[:, 0:128],
                        lhsT=khi[h // 2][64 * (h % 2):64 * (h % 2) + 64, :],
                        rhs=qhi[h // 2][64 * (h % 2):64 * (h % 2) + 64, :],
                        start=True, stop=True,
                    )
                    nc.vector.tensor_add(shi[:, 0:128], shi[:, 0:128], U[:])
                    nc.scalar.activation(out=pthi[:, h, :], in_=shi[:, 0:128],
                                         func=Exp, scale=0.125)

            def emit_hi_av():
                pohi = shp.tile([65, 4, 128], f32, tag="pp", name="pohi")
                for h in range(4):
                    nc.tensor.matmul(
                        pohi[:, h, :], lhsT=vthi[:, h, :], rhs=pthi[:, h, :],
                        start=(h == 0), stop=(h == 3),
                    )
                for h in range(4):
                    li = lpool.tile([1, 128], f32, tag="li", name="hli")
                    nc.vector.reciprocal(out=li[:], in_=pohi[64:65, h, :])
                    lb = lpool.tile([64, 128], f32, tag="lb", name="hlb")
                    nc.gpsimd.partition_broadcast(lb[:], li[:])
                    hb = 64 * (h % 2)
                    nc.vector.tensor_mul(
                        outTn[h // 2][hb:hb + 64, 0:128], pohi[0:64, h, :], lb[:]
                    )

            # ---------------- fp8 projections, per 512-chunk ----------------
            xts, tbs = {}, {}

            def emit_loads(p):
                if p >= npc or p in xts:
                    return
                xt = p1x.tile([128, 8, PC], f8, tag="xt")
                nc.sync.dma_start(out=xt[:], in_=xt8[p])
                tb = p1x.tile([128, 2, PC], bf16, tag="tb")
                nc.sync.dma_start(out=tb[:], in_=tab[p])
                xts[p], tbs[p] = xt, tb

            def emit_proj_kt(p, kt):
                """Q and K projection + rope for head-pair kt of chunk p,
                phase-interleaved (both PSUM copies run while the swap DMAs
                fly, so the table-multiplies rarely stall).  High priority:
                this chain gates whole chunks of attention."""
                xt, tb = xts[p], tbs[p]
                csl = slice(p * PC, p * PC + PC)
                pps, praws, swps = [], [], []
                for w_s in (wq_s, wk_s):
                    # one PSUM group across both 256-wide halves (shared 2KB
                    # zero region: start once, stop on the final matmul)
                    pp = shp.tile([128, 512], f32, tag="pp")
                    for hf in range(2):
                        for a in range(4):
                            nc.tensor.matmul(
                                pp[:, hf * 256:hf * 256 + 256],
                                lhsT=w_s[:, 2 * a:2 * a + 2, kt * 128:kt * 128 + 128],
                                rhs=xt[:, 2 * a:2 * a + 2, hf * 256:hf * 256 + 256],
                                start=(hf == 0 and a == 0),
                                stop=(hf == 1 and a == 3),
                                perf_mode=DR, skip_group_check=True,
                            )
                    pps.append(pp)
                for i in range(2):
                    praw = rtmp.tile([128, PC], bf16, tag="praw")
                    nc.vector.tensor_copy(out=praw[:], in_=pps[i][:])
                    praws.append(praw)
                    swp = rtmp.tile([128, PC], bf16, tag="swp")
                    nc.sync.dma_start(out=swp[0:64, :], in_=praw[64:128, :])
                    nc.sync.dma_start(out=swp[64:128, :], in_=praw[0:64, :])
                    swps.append(swp)
                for i, rot in enumerate((qrot, krot)):
                    pc_ = rtmp.tile([128, PC], bf16, tag="pc")
                    ps_ = rtmp.tile([128, PC], bf16, tag="ps")
                    nc.vector.tensor_mul(pc_[:], praws[i][:], tb[:, 0, :])
                    nc.vector.tensor_mul(ps_[:], swps[i][:], tb[:, 1, :])
                    # heads (2kt, 2kt+1) live in tile kt on partitions 0-63;
                    # plane t0=even-rot, t1=odd-rot.
                    # praw rows: [h0ev h1ev | h0od h1od].
                    nc.gpsimd.tensor_add(
                        rot[kt][:, 0, csl], pc_[0:64, :], ps_[0:64, :])
                    nc.gpsimd.tensor_add(
                        rot[kt][:, 1, csl], pc_[64:128, :], ps_[64:128, :])

            def emit_proj_v(p, half):
                """V projection for 2 of chunk p's 4 position-blocks."""
                xt = xts[p]
                for ibl in (2 * half, 2 * half + 1):
                    ib = p * (PC // 128) + ibl
                    vp = shp.tile([128, KSLICE], f32, tag="pp", name="vp")
                    for a in range(4):
                        nc.tensor.matmul(
                            vp[:],
                            lhsT=xt[:, 2 * a:2 * a + 2, ibl * 128:ibl * 128 + 128],
                            rhs=wv_s[:, 2 * a:2 * a + 2, :],
                            start=(a == 0), stop=(a == 3),
                            perf_mode=DR, skip_group_check=True,
                        )
                    nc.scalar.copy(
                        out=vt[:, ib // 2, :, ib % 2, 0:64],
                        in_=vp[:].rearrange("p (h z) -> p h z", z=64),
                    )

            def emit_proj(p):
                emit_proj_kt(p, 0)
                emit_proj_kt(p, 1)
                emit_proj_v(p, 0)
                emit_proj_v(p, 1)

            emit_loads(0)
            emit_loads(1)
            nc.sync.dma_start(out=wv_s[:], in_=wv8[:])
            nc.sync.dma_start(out=tm_s[:], in_=tmsk[:])
            nc.sync.dma_start(out=ni_s[:], in_=negi[:])
            emit_proj(0)

            def emit_hi_loads():
                nc.sync.dma_start(out=xhi_s[:], in_=xhi[:])
                nc.sync.dma_start(out=whq_s[:], in_=whq[:])
                nc.sync.dma_start(out=whk_s[:], in_=whk[:])
                nc.sync.dma_start(out=whv_s[:], in_=whv[:])
                nc.sync.dma_start(out=htab_s[:], in_=htab[:])

            def emit_wo_load():
                nc.sync.dma_start(out=wo_s[:], in_=woT[:].rearrange("q p d -> p q d"))

            # ---------------- attention: pipelined (chunk, head) units -------
            def make_unit(c, h):
                """Returns (ngrp, emit_sg, emit_expav) closures for one
                attention unit: chunk c (q-window), head h."""
                q0 = c * CH + (128 if c == 0 else 0)   # hi-path covers rows 0-127
                W = c * CH + CH - q0
                njb = 2 * (c + 1)                       # causal key blocks
                ngrp = (njb + 3) // 4
                hp = slice(32 * (h % 2), 32 * (h % 2) + 32)
                qr, kr = qrot[h // 2], krot[h // 2]
                st = {"po": None, "sg": {}, "pt": {}}

                def emit_sg(G):
                    gn = min(4, njb - 4 * G)
                    sg = sgp.tile([128, 4, 256], f32, tag="sg")
                    for rj in range(0, gn, 2):      # per 2KB psum region
                        jA, jB = 4 * G + rj, 4 * G + rj + 1
                        mms = [
                            (sg[:, rj + s, 0:W],
                             kr[hp, :, j * 128:j * 128 + 128],
                             qr[hp, :, q0:q0 + W])
                            for s, j in ((0, jA), (1, jB))
                        ]
                        for s, j in ((0, jA), (1, jB)):
                            # diagonal-block causal mask matmul
                            if j >= 2 * c and not (c == 0 and j == 0):
                                which = j - 2 * c   # 0: T0, 1: T1 pattern
                                mms.append(
                                    (sg[:, rj + s, 0:W], ni_s[:],
                                     tm_s[:, :, which, CH - W:CH]))
                        for i, (o, l, r) in enumerate(mms):
                            nc.tensor.matmul(
                                o, lhsT=l, rhs=r,
                                start=(i == 0), stop=(i == len(mms) - 1),
                                perf_mode=DR, skip_group_check=True,
                            )
                    st["sg"][G] = sg

                def emit_expav(G):
                    gn = min(4, njb - 4 * G)
                    pt = ptpool.tile([128, 4, 256], f8, tag="pt")
                    nc.scalar.activation(
                        out=pt[:, 0:gn, 0:W], in_=st["sg"].pop(G)[:, 0:gn, 0:W],
                        func=Exp, scale=EXP_SCALE,
                    )
                    if st["po"] is None:
                        st["po"] = pop.tile([65, 512], f32, tag="po", name="po")
                    po = st["po"]
                    for u in range(0, gn, 2):
                        jb = 4 * G + u
                        nc.tensor.matmul(
                            po[:, 0:W],
                            lhsT=vt[:, jb // 2, h, :, 0:65],
                            rhs=pt[:, u:u + 2, 0:W],
                            start=(jb == 0), stop=(jb + 2 >= njb),
                            perf_mode=DR, skip_group_check=True,
                        )
                    if G == ngrp - 1:
                        # epilogue: normalize by the 16.0-ones denominator
                        # row.  High priority: the po PSUM pool recycles
                        # through this chain, so a lagging epilogue stalls
                        # the AV accumulation two units later.
                        with tc.high_priority():
                            li = lpool.tile([1, 256], f32, tag="li")
                            nc.vector.reciprocal(out=li[:, 0:W], in_=po[64:65, 0:W])
                            lb = lpool.tile([64, 256], f32, tag="lb")
                            nc.gpsimd.partition_broadcast(lb[:, 0:W], li[:, 0:W])
                            hb = 64 * (h % 2)
                            nc.vector.tensor_mul(
                                outTn[h // 2][hb:hb + 64, q0:q0 + W], po[0:64, 0:W],
                                lb[:, 0:W],
                            )

                return ngrp, emit_sg, emit_expav

            def emit_oproj(c, ib):
                ys = yst.tile([128, D_MODEL], f32, tag="ys")
                for ns in range(2):
                    yp = shp.tile([128, 512], f32, tag="pp", name="yp")
                    for pr in range(2):
                        nc.tensor.matmul(
                            yp[:],
                            lhsT=outTn[pr][:, ib * 128:ib * 128 + 128],
                            rhs=wo_s[:, pr, ns * 512:ns * 512 + 512],
                            start=(pr == 0), stop=(pr == 1),
                        )
                    nc.vector.tensor_copy(
                        out=ys[:, ns * 512:ns * 512 + 512], in_=yp[:])
                nc.sync.dma_start(
                    out=y[ib * 128:ib * 128 + 128, :], in_=ys[:])

            # Aux PE work is interleaved at unit boundaries.  Projections are
            # front-loaded (the Activation engine idles until later chunks'
            # scores exist, so finishing all projections early flattens the
            # causal-triangular exp schedule); o_proj items fill afterwards.
            from collections import deque

            projq = deque()                         # (proj_idx, closure)
            for p in (1, 2, 3):
                if p >= 2:
                    projq.append((p, lambda p=p: emit_loads(p)))
                projq.append((p, lambda p=p: emit_proj_kt(p, 0)))
                projq.append((p, lambda p=p: emit_proj_kt(p, 1)))
                projq.append((p, lambda p=p: emit_proj_v(p, 0)))
                projq.append((p, lambda p=p: emit_proj_v(p, 1)))
                if p == 1:
                    for fn in (emit_hi_loads, lambda: emit_hi_qk(0),
                               lambda: emit_hi_qk(1), emit_hi_v, emit_hi_s,
                               emit_hi_av, emit_wo_load):
                        projq.append((1, fn))
            oprojq = deque()                        # ready o_proj items

            def drain_proj(upto):
                while projq and projq[0][0] <= upto:
                    projq.popleft()[1]()

            pending = deque()                       # (emit_expav, G)
            for c in range(nch):
                drain_proj(c // 2)                  # hard dependency
                for h in range(HEADS_PER_CORE):
                    ngrp, emit_sg, emit_expav = make_unit(c, h)
                    for G in range(ngrp):
                        emit_sg(G)
                        pending.append((emit_expav, G))
                        while len(pending) > 1:
                            f, g = pending.popleft()
                            f(g)
                        # boundary aux: prefer projections, two per slot.
                        # none during chunk 0 — early aux wedges the in-order
                        # PE queue behind not-yet-loaded x chunks.
                        for _ in range(2):
                            if projq:
                                projq.popleft()[1]()
                            elif oprojq:
                                oprojq.popleft()()
                if c < nch - 1:
                    oprojq.append(lambda c=c: emit_oproj(c, 2 * c))
                    oprojq.append(lambda c=c: emit_oproj(c, 2 * c + 1))
            while pending:
                f, g = pending.popleft()
                f(g)
            while oprojq:
                oprojq.popleft()()
            emit_oproj(nch - 1, 2 * nch - 2)
            emit_oproj(nch - 1, 2 * nch - 1)

    nc.finalize()
    return nc


def _rope_tables(seq, width, swapped_sign_rows):
    """cos/sin tables in [128, seq] row layout; freq index = row mod 32.
    swapped_sign_rows: '64' -> rows 0-63 get -sin (64-swap layout);
    '32' -> rows [32:64] and [96:128] get -sin (32-swap layout)."""
    half = HEAD_DIM // 2
    inv = 1.0 / (THETA ** (2.0 * np.arange(half) / HEAD_DIM))
    ang = np.arange(seq, dtype=np.float64)[:, None] * inv[None, :]  # [seq, 32]
    cos32 = np.cos(ang).T  # [32, seq]
    sin32 = np.sin(ang).T
    cosI = np.tile(cos32, (4, 1))
    if swapped_sign_rows == "64":
        sinI = np.concatenate([-np.tile(sin32, (2, 1)), np.tile(sin32, (2, 1))], 0)
    else:
        # 32-swap layout: swp rows [od, ev, od', ev'] -> signs [-,+,-,+]
        sinI = np.concatenate([-sin32, sin32, -sin32, sin32], 0)
    return cosI[:, :width], sinI[:, :width]


def make_in_maps(in_features, q_proj, k_proj, v_proj, o_proj, token_positions,
                 seq=SEQ):
    x = np.asarray(in_features, np.float32)
    wq = np.asarray(q_proj, np.float32)
    wk = np.asarray(k_proj, np.float32)
    wv = np.asarray(v_proj, np.float32)
    wo = np.asarray(o_proj, np.float32)

    # fp8-path q/k row perm per 128-row ktile: [h0ev(32) h1ev(32) h0od(32) h1od(32)]
    ev = np.arange(0, HEAD_DIM, 2)
    od = np.arange(1, HEAD_DIM, 2)
    perm8 = []
    for kt in range(2):
        h0, h1 = 2 * kt, 2 * kt + 1
        perm8 += [h0 * HEAD_DIM + ev, h1 * HEAD_DIM + ev,
                  h0 * HEAD_DIM + od, h1 * HEAD_DIM + od]
    perm8 = np.concatenate(perm8)  # local perm within a core's 256 rows
    # hi-path perm: [h0ev h0od h1ev h1od]
    permh = []
    for kt in range(2):
        h0, h1 = 2 * kt, 2 * kt + 1
        permh += [h0 * HEAD_DIM + ev, h0 * HEAD_DIM + od,
                  h1 * HEAD_DIM + ev, h1 * HEAD_DIM + od]
    permh = np.concatenate(permh)

    npc = seq // PC
    # x chunked fp8: [npc, 128, 8, PC]
    xt8b, xhib = [], []
    for b in range(x.shape[0]):
        xT = np.ascontiguousarray(x[b].T)                       # [1024, seq]
        xt8b.append(np.ascontiguousarray(
            xT.reshape(8, 128, npc, PC).transpose(2, 1, 0, 3)).astype(F8))
        xhib.append(np.ascontiguousarray(
            xT[:, 0:128].reshape(8, 128, 128).transpose(1, 0, 2)).astype(BF16))

    # rope tables, fp8 path (64-swap): [npc, 128, 2, PC]
    cosI, sinI = _rope_tables(seq, seq, "64")
    tabf = np.stack([cosI, sinI], axis=1)                       # [128, 2, seq]
    tabf = np.ascontiguousarray(
        tabf.reshape(128, 2, npc, PC).transpose(2, 0, 1, 3)).astype(BF16)
    # hi tables (32-swap), width 128
    cosH, sinH = _rope_tables(seq, 128, "32")
    htabf = np.ascontiguousarray(np.stack([cosH, sinH], 1)).astype(BF16)

    # causal mask patterns T0/T1 [128, CH], value MASKV
    k_ = np.arange(128)[:, None]
    j_ = np.arange(CH)[None, :]
    T0 = (j_ < k_).astype(np.float32) * MASKV
    T1 = (j_ < k_ + 128).astype(np.float32) * MASKV
    tmskf = np.ascontiguousarray(
        np.broadcast_to(np.stack([T0, T1], 0)[None], (2, 2, 128, CH))
        .transpose(2, 0, 1, 3)).astype(F8)
    negif = np.ascontiguousarray(
        np.broadcast_to((-MASKV * np.eye(128, dtype=np.float32))[:, None, :],
                        (128, 2, 128))).astype(F8)

    def wtile(w, perm, scale, dtype):
        # rows ks (already core-sliced, perm applied), transposed -> [128, 8, 256]
        wT = np.ascontiguousarray(w[perm].T)                    # [1024, 256]
        return np.ascontiguousarray(
            (wT * scale).reshape(8, 128, KSLICE).transpose(1, 0, 2)).astype(dtype)

    in_maps = []
    for core in range(N_CORES):
        b, g = divmod(core, HEADS_PER_CORE)
        ks = np.arange(g * KSLICE, (g + 1) * KSLICE)
        wq_c, wk_c, wv_c = wq[ks], wk[ks], wv[ks]
        in_maps.append({
            "xt8": xt8b[b],
            "wq8": wtile(wq_c, perm8, WSCALE, F8),
            "wk8": wtile(wk_c, perm8, WSCALE, F8),
            "wv8": wtile(wv_c, np.arange(KSLICE), WSCALE, F8),
            "tab": tabf,
            "tmsk": tmskf,
            "negi": negif,
            "woT": np.ascontiguousarray(
                wo[:, ks].T.reshape(2, 128, D_MODEL)).astype(np.float32),
            "xhi": xhib[b],
            "whq": wtile(wq_c, permh, 1.0, BF16),
            "whk": wtile(wk_c, permh, 1.0, BF16),
            "whv": wtile(wv_c, np.arange(KSLICE), 1.0, BF16),
            "htab": htabf,
        })
    return in_maps


def assemble(results, batch=2):
    ys = []
    for b in range(batch):
        parts = [results[b * HEADS_PER_CORE + g]["y"].astype(np.float64)
                 for g in range(HEADS_PER_CORE)]
        ys.append(np.sum(parts, axis=0, dtype=np.float64).astype(np.float32))
    return np.stack(ys, axis=0)


_NC_CACHE = {}


def get_nc(seq=SEQ):
    import os
    key = (seq, os.environ.get("KBISECT", "3"))
    if key not in _NC_CACHE:
        _NC_CACHE[key] = build_nc(seq)
    return _NC_CACHE[key]


def kernel(**inputs):
    from concourse.bass_utils import run_bass_kernel_spmd

    nc = get_nc()
    in_maps = make_in_maps(**inputs)
    res = run_bass_kernel_spmd(nc, in_maps, list(range(N_CORES)))
    return assemble(res.results)


if __name__ == "__main__":
    rng = np.random.default_rng(0)
    ins = {
        "in_features": rng.standard_normal((2, SEQ, D_MODEL), np.float32),
        "q_proj": (rng.standard_normal((D_MODEL, D_MODEL)) * 0.02).astype(np.float32),
        "k_proj": (rng.standard_normal((D_MODEL, D_MODEL)) * 0.02).astype(np.float32),
        "v_proj": (rng.standard_normal((D_MODEL, D_MODEL)) * 0.02).astype(np.float32),
        "o_proj": (rng.standard_normal((D_MODEL, D_MODEL)) * 0.02).astype(np.float32),
        "token_positions": np.arange(SEQ, dtype=np.int32),
    }
    out = kernel(**ins)
    print("kernel output:", out.shape, out.dtype)


# revision 60
# speedup vs baseline: 1.0565x; 1.0372x over previous
"""Causal multi-head attention (RoPE) on 8 Trainium2 NeuronCores — fp8 edition.

Sharding: (batch=2) x (head groups=4) -> 8 cores; core c = 4*b + g handles
batch b, heads [4g, 4g+4). Each core computes its 4 heads' attention plus its
partial o_proj contribution; the host sums 4 partials per batch.

Per-core kernel (single Tile program, SPMD over cores):
  - QKV projections and the S / AV attention matmuls run in fp8e4m3 with
    MatmulPerfMode.DoubleRow (2 contraction tiles per instruction at 0.5
    cycles/row -> 4x the f32r/bf16 matmul throughput).  Weights are
    pre-scaled by 16 on the host to center them in fp8 range; the scale is
    folded into the exp() activation scale (2^-11) and a 16.0 ones-column
    that yields the softmax denominator.
  - Queries/keys 0-127 run through a separate bf16 "hi-precision" path
    (fp8 noise does not average out over few softmax terms at early rows).
  - Causal masking of diagonal key-blocks is done on the PE: a constant
    fp8 DoubleRow matmul accumulates -115200 (* 2^-11 -> -56) into masked
    score positions, so exp() flushes them to exact zero.  No vector-engine
    mask pass.
  - o_proj stays f32r (accuracy); softmax epilogue = reciprocal (DVE) +
    partition_broadcast (Pool) + multiply (DVE).
  - RoPE: PSUM->SBUF bf16 copy (DVE), 64-partition swap via SBUF-SBUF DMA,
    bf16 table multiplies (DVE 2x mode), two adds -> fp8 (Pool).
  - Emission is software-pipelined: the 32 (chunk, head) attention units
    form one stream with one score-group of lookahead, and projection /
    hi-path / o_proj work is spread across unit boundaries (projections
    front-loaded) so the Activation engine — the critical engine at ~78 us
    of exp() — stays fed.  Hardware quirks found on the way: GPSIMD cannot
    touch PSUM; a PSUM accumulation bracket must keep one tile_position
    (mixing base partitions crashes the device); DoubleRow Ldweights needs
    a >=128-element pair stride.

Layouts (per core):
  qrot/krot: two [64, 2, seq] fp8 tiles (head pair kt); head h on
    partitions 32(h%2)..+32, plane t in {0,1} = rotated-even/odd dims;
    S = DoubleRow over (32 partitions x 2 planes) = the 64-dim contraction.
  vt [128, 8, 4, 2, 128] fp8: key-block pair, head, block-in-pair; cols
    0:64 = v-dims, col 64 = 16.0 ones column (softmax denominator).
  S^T tiles [128 keys, W queries] accumulate 4 key-blocks per PSUM group;
  one exp() per group feeds DoubleRow AV over key-block pairs.
"""

import sys

for _p in ("/opt/trn_rl_repo",):
    if _p not in sys.path:
        sys.path.insert(0, _p)

import numpy as np
import ml_dtypes

F8 = ml_dtypes.float8_e4m3
BF16 = ml_dtypes.bfloat16

SEQ = 2048
D_MODEL = 1024
NUM_HEADS = 16
HEAD_DIM = 64
THETA = 10000.0
N_CORES = 8
HEADS_PER_CORE = 4
KSLICE = HEADS_PER_CORE * HEAD_DIM  # 256 projection rows per core
PC = 512          # projection chunk width (4 chunks)
CH = 256          # attention query-chunk width (8 chunks)
WSCALE = 16.0     # host pre-scale on wq/wk/wv (fp8 range centering)
EXP_SCALE = 0.125 / (WSCALE * WSCALE)  # folds q*k scale^2 and 1/sqrt(hd)
MASKV = 240.0     # fp8e4m3 max-ish; DR mask adds -2*240*240 = -115200


def build_nc(seq=SEQ):
    import os
    BISECT = int(os.environ.get("KBISECT", "3"))  # 1: proj only, 2: +S/exp, 3: full
    KHI = int(os.environ.get("KHI", "1"))
    KHIPART = int(os.environ.get("KHIPART", "4"))
    KPROJ = int(os.environ.get("KPROJ", "1"))
    KOPROJ = int(os.environ.get("KOPROJ", "1"))
    import concourse.mybir as mybir
    import concourse.tile as tile
    from concourse import bacc
    from contextlib import ExitStack

    f32 = mybir.dt.float32
    f32r = mybir.dt.float32r
    bf16 = mybir.dt.bfloat16
    f8 = mybir.dt.float8e4
    DR = mybir.MatmulPerfMode.DoubleRow
    Exp = mybir.ActivationFunctionType.Exp

    npc = seq // PC              # 4 projection chunks
    nch = seq // CH              # 8 attention chunks
    nib = seq // 128             # 16 key/row blocks

    nc = bacc.Bacc(None, target_bir_lowering=False)

    # fp8 path inputs
    xt8 = nc.declare_dram_parameter("xt8", [npc, 128, 8, PC], f8, isOutput=False)
    wq8 = nc.declare_dram_parameter("wq8", [128, 8, KSLICE], f8, isOutput=False)
    wk8 = nc.declare_dram_parameter("wk8", [128, 8, KSLICE], f8, isOutput=False)
    wv8 = nc.declare_dram_parameter("wv8", [128, 8, KSLICE], f8, isOutput=False)
    tab = nc.declare_dram_parameter("tab", [npc, 128, 2, PC], bf16, isOutput=False)
    tmsk = nc.declare_dram_parameter("tmsk", [128, 2, 2, CH], f8, isOutput=False)
    negi = nc.declare_dram_parameter("negi", [128, 2, 128], f8, isOutput=False)
    # o_proj (f32r)
    woT = nc.declare_dram_parameter("woT", [2, 128, D_MODEL], f32r, isOutput=False)
    # hi-precision path (rows/keys 0-127), bf16
    xhi = nc.declare_dram_parameter("xhi", [128, 8, 128], bf16, isOutput=False)
    whq = nc.declare_dram_parameter("whq", [128, 8, KSLICE], bf16, isOutput=False)
    whk = nc.declare_dram_parameter("whk", [128, 8, KSLICE], bf16, isOutput=False)
    whv = nc.declare_dram_parameter("whv", [128, 8, KSLICE], bf16, isOutput=False)
    htab = nc.declare_dram_parameter("htab", [128, 2, 128], bf16, isOutput=False)

    y = nc.declare_dram_parameter("y", [seq, D_MODEL], f32, isOutput=True)

    with tile.TileContext(nc) as tc, ExitStack() as ctx:
        persist = ctx.enter_context(tc.tile_pool(name="persist", bufs=1))

        qrot = [persist.tile([64, 2, seq], f8, tag=f"qrot{k}", name=f"qrot{k}")
                for k in range(2)]
        krot = [persist.tile([64, 2, seq], f8, tag=f"krot{k}", name=f"krot{k}")
                for k in range(2)]
        # AV DoubleRow weights: k-block PAIR on dim3 with 128-wide inner
        # stride (walrus ISA check rejects <128 pair strides); cols 0:64 =
        # v-dims, col 64 = 16.0 ones (softmax denominator), 65:128 unused.
        vt = persist.tile([128, nib // 2, HEADS_PER_CORE, 2, 128], f8, tag="vt")
        outTn = [persist.tile([128, seq], f32r, tag=f"outTnP{p}", name=f"outTnP{p}")
                 for p in range(HEADS_PER_CORE // 2)]
        nc.vector.memset(vt[:, :, :, :, 64:65], WSCALE)

        wq_s = persist.tile([128, 8, KSLICE], f8, tag="wq_s")
        wk_s = persist.tile([128, 8, KSLICE], f8, tag="wk_s")
        wv_s = persist.tile([128, 8, KSLICE], f8, tag="wv_s")
        wo_s = persist.tile([128, 2, D_MODEL], f32r, tag="wo_s")
        tm_s = persist.tile([128, 2, 2, CH], f8, tag="tm_s")
        ni_s = persist.tile([128, 2, 128], f8, tag="ni_s")
        # load order = dependency order: first projection chunk's x and the
        # q/k weights gate everything; wo/o_proj and hi-path gear come later.
        nc.sync.dma_start(out=wq_s[:], in_=wq8[:])
        nc.sync.dma_start(out=wk_s[:], in_=wk8[:])

        # hi-path persistent
        xhi_s = persist.tile([128, 8, 128], bf16, tag="xhi_s")
        whq_s = persist.tile([128, 8, KSLICE], bf16, tag="whq_s")
        whk_s = persist.tile([128, 8, KSLICE], bf16, tag="whk_s")
        whv_s = persist.tile([128, 8, KSLICE], bf16, tag="whv_s")
        htab_s = persist.tile([128, 2, 128], bf16, tag="htab_s")
        qhi = [persist.tile([128, 128], bf16, tag=f"qhi{k}", name=f"qhi{k}") for k in range(2)]
        khi = [persist.tile([128, 128], bf16, tag=f"khi{k}", name=f"khi{k}") for k in range(2)]
        vthi = persist.tile([128, HEADS_PER_CORE, 65], bf16, tag="vthi")
        nc.vector.memset(vthi[:, :, 64:65], 1.0)

        with (
            tc.tile_pool(name="p1x", bufs=2) as p1x,
            tc.tile_pool(name="rtmp", bufs=8) as rtmp,
            tc.tile_pool(name="ptpool", bufs=4) as ptpool,
            tc.tile_pool(name="lpool", bufs=6) as lpool,
            tc.tile_pool(name="ystage", bufs=4) as yst,
            tc.tile_pool(name="sgpsum", bufs=2, space="PSUM") as sgp,
            tc.tile_pool(name="popsum", bufs=2, space="PSUM") as pop,
            tc.tile_pool(name="shpsum", bufs=2, space="PSUM") as shp,
        ):
            if BISECT < 3 or not KHI:
                zt = rtmp.tile([128, 512], f32, tag="zfill")
                nc.vector.memset(zt[:], 0.0)
                for p_ in range(2):
                    for cc in range(4):
                        nc.vector.tensor_copy(
                            out=outTn[p_][:, cc * 512:cc * 512 + 512], in_=zt[:])

            # ---------------- hi-precision path: rows/keys 0-127 ------------
            # Emitted as aux items interleaved with early attention chunks
            # (its long serial chain would otherwise stall the pipe start).
            U = persist.tile([128, 128], f32, tag="umask")
            nc.gpsimd.memset(U[:], 0.0)
            nc.gpsimd.affine_select(
                out=U[:], in_=U[:], compare_op=mybir.AluOpType.is_ge,
                fill=-1e9, base=0, pattern=[[1, 128]], channel_multiplier=-1,
            )
            pthi = ptpool.tile([128, 4, 128], bf16, tag="pthi")

            def emit_hi_qk(kt):
                ctxh = tc.high_priority(offset=500)
                ctxh.__enter__()
                # projections (bf16, contraction d=1024 over 8 d-tiles)
                for w_s, dst in ((whq_s, qhi[kt]), (whk_s, khi[kt])):
                    pp = shp.tile([128, 512], f32, tag="pp", name=f"hiqk{kt}")
                    for dt in range(8):
                        nc.tensor.matmul(
                            pp[:, 0:128],
                            lhsT=w_s[:, dt, kt * 128:kt * 128 + 128],
                            rhs=xhi_s[:, dt, :],
                            start=(dt == 0), stop=(dt == 7),
                        )
                    # rope (baseline-perm rows: [h0ev h0od h1ev h1od], 32-swap)
                    praw = rtmp.tile([128, 128], bf16, tag="hpraw")
                    nc.vector.tensor_copy(out=praw[:], in_=pp[:, 0:128])
                    swp = rtmp.tile([128, 128], bf16, tag="hswp")
                    for blk in range(4):
                        d, s = blk * 32, blk * 32 + (32 if blk % 2 == 0 else -32)
                        nc.vector.tensor_copy(out=swp[d:d + 32, :], in_=praw[s:s + 32, :])
                    pc_ = rtmp.tile([128, 128], bf16, tag="hpc")
                    ps_ = rtmp.tile([128, 128], bf16, tag="hps")
                    nc.gpsimd.tensor_mul(pc_[:], praw[:], htab_s[:, 0, :])
                    nc.gpsimd.tensor_mul(ps_[:], swp[:], htab_s[:, 1, :])
                    nc.vector.tensor_add(dst[:], pc_[:], ps_[:])
                ctxh.__exit__(None, None, None)

            def emit_hi_v():
                vp = shp.tile([128, KSLICE], f32, tag="pp", name="hiv")
                for dt in range(8):
                    nc.tensor.matmul(
                        vp[:], lhsT=xhi_s[:, dt, :], rhs=whv_s[:, dt, :],
                        start=(dt == 0), stop=(dt == 7),
                    )
                nc.scalar.copy(
                    out=vthi[:, :, 0:64],
                    in_=vp[:].rearrange("p (h z) -> p h z", z=64),
                )

            def emit_hi_s():
                # NB: one accumulation bracket per head — mixing tile_position
                # (base partition) inside a PSUM bracket crashes the hardware.
                for h in range(4):
                    shi = shp.tile([128, 512], f32, tag="pp", name=f"shi{h}")
                    nc.tensor.matmul(
                        shi[:, 0:128],
                        lhsT=khi[h // 2][64 * (h % 2):64 * (h % 2) + 64, :],
                        rhs=qhi[h // 2][64 * (h % 2):64 * (h % 2) + 64, :],
                        start=True, stop=True,
                    )
                    nc.vector.tensor_add(shi[:, 0:128], shi[:, 0:128], U[:])
                    nc.scalar.activation(out=pthi[:, h, :], in_=shi[:, 0:128],
                                         func=Exp, scale=0.125)

            def emit_hi_av():
                pohi = shp.tile([65, 4, 128], f32, tag="pp", name="pohi")
                for h in range(4):
                    nc.tensor.matmul(
                        pohi[:, h, :], lhsT=vthi[:, h, :], rhs=pthi[:, h, :],
                        start=(h == 0), stop=(h == 3),
                    )
                for h in range(4):
                    li = lpool.tile([1, 128], f32, tag="li", name="hli")
                    nc.vector.reciprocal(out=li[:], in_=pohi[64:65, h, :])
                    lb = lpool.tile([64, 128], f32, tag="lb", name="hlb")
                    nc.gpsimd.partition_broadcast(lb[:], li[:])
                    hb = 64 * (h % 2)
                    nc.vector.tensor_mul(
                        outTn[h // 2][hb:hb + 64, 0:128], pohi[0:64, h, :], lb[:]
                    )

            # ---------------- fp8 projections, per 512-chunk ----------------
            xts, tbs = {}, {}

            def emit_loads(p):
                if p >= npc or p in xts:
                    return
                xt = p1x.tile([128, 8, PC], f8, tag="xt")
                nc.sync.dma_start(out=xt[:], in_=xt8[p])
                tb = p1x.tile([128, 2, PC], bf16, tag="tb")
                nc.sync.dma_start(out=tb[:], in_=tab[p])
                xts[p], tbs[p] = xt, tb

            def emit_proj_kt(p, kt):
                """Q and K projection + rope for head-pair kt of chunk p,
                phase-interleaved (both PSUM copies run while the swap DMAs
                fly, so the table-multiplies rarely stall).  High priority:
                this chain gates whole chunks of attention."""
                xt, tb = xts[p], tbs[p]
                csl = slice(p * PC, p * PC + PC)
                pps, praws, swps = [], [], []
                for w_s in (wq_s, wk_s):
                    # one PSUM group across both 256-wide halves (shared 2KB
                    # zero region: start once, stop on the final matmul)
                    pp = shp.tile([128, 512], f32, tag="pp")
                    for hf in range(2):
                        for a in range(4):
                            nc.tensor.matmul(
                                pp[:, hf * 256:hf * 256 + 256],
                                lhsT=w_s[:, 2 * a:2 * a + 2, kt * 128:kt * 128 + 128],
                                rhs=xt[:, 2 * a:2 * a + 2, hf * 256:hf * 256 + 256],
                                start=(hf == 0 and a == 0),
                                stop=(hf == 1 and a == 3),
                                perf_mode=DR, skip_group_check=True,
                            )
                    pps.append(pp)
                for i in range(2):
                    praw = rtmp.tile([128, PC], bf16, tag="praw")
                    nc.vector.tensor_copy(out=praw[:], in_=pps[i][:])
                    praws.append(praw)
                    swp = rtmp.tile([128, PC], bf16, tag="swp")
                    nc.sync.dma_start(out=swp[0:64, :], in_=praw[64:128, :])
                    nc.sync.dma_start(out=swp[64:128, :], in_=praw[0:64, :])
                    swps.append(swp)
                for i, rot in enumerate((qrot, krot)):
                    pc_ = rtmp.tile([128, PC], bf16, tag="pc")
                    ps_ = rtmp.tile([128, PC], bf16, tag="ps")
                    nc.vector.tensor_mul(pc_[:], praws[i][:], tb[:, 0, :])
                    nc.vector.tensor_mul(ps_[:], swps[i][:], tb[:, 1, :])
                    # heads (2kt, 2kt+1) live in tile kt on partitions 0-63;
                    # plane t0=even-rot, t1=odd-rot.
                    # praw rows: [h0ev h1ev | h0od h1od].
                    nc.gpsimd.tensor_add(
                        rot[kt][:, 0, csl], pc_[0:64, :], ps_[0:64, :])
                    nc.gpsimd.tensor_add(
                        rot[kt][:, 1, csl], pc_[64:128, :], ps_[64:128, :])

            def emit_proj_v(p, half):
                """V projection for 2 of chunk p's 4 position-blocks."""
                xt = xts[p]
                for ibl in (2 * half, 2 * half + 1):
                    ib = p * (PC // 128) + ibl
                    vp = shp.tile([128, KSLICE], f32, tag="pp", name="vp")
                    for a in range(4):
                        nc.tensor.matmul(
                            vp[:],
                            lhsT=xt[:, 2 * a:2 * a + 2, ibl * 128:ibl * 128 + 128],
                            rhs=wv_s[:, 2 * a:2 * a + 2, :],
                            start=(a == 0), stop=(a == 3),
                            perf_mode=DR, skip_group_check=True,
                        )
                    nc.scalar.copy(
                        out=vt[:, ib // 2, :, ib % 2, 0:64],
                        in_=vp[:].rearrange("p (h z) -> p h z", z=64),
                    )

            def emit_proj(p):
                emit_proj_kt(p, 0)
                emit_proj_kt(p, 1)
                emit_proj_v(p, 0)
                emit_proj_v(p, 1)

            emit_loads(0)
            emit_loads(1)
            nc.sync.dma_start(out=wv_s[:], in_=wv8[:])
            nc.sync.dma_start(out=tm_s[:], in_=tmsk[:])
            nc.sync.dma_start(out=ni_s[:], in_=negi[:])
            emit_proj(0)

            def emit_hi_loads():
                nc.sync.dma_start(out=xhi_s[:], in_=xhi[:])
                nc.sync.dma_start(out=whq_s[:], in_=whq[:])
                nc.sync.dma_start(out=whk_s[:], in_=whk[:])
                nc.sync.dma_start(out=whv_s[:], in_=whv[:])
                nc.sync.dma_start(out=htab_s[:], in_=htab[:])

            def emit_wo_load():
                nc.sync.dma_start(out=wo_s[:], in_=woT[:].rearrange("q p d -> p q d"))

            # ---------------- attention: pipelined (chunk, head) units -------
            def make_unit(c, h):
                """Returns (ngrp, emit_sg, emit_expav) closures for one
                attention unit: chunk c (q-window), head h."""
                q0 = c * CH + (128 if c == 0 else 0)   # hi-path covers rows 0-127
                W = c * CH + CH - q0
                njb = 2 * (c + 1)                       # causal key blocks
                ngrp = (njb + 3) // 4
                hp = slice(32 * (h % 2), 32 * (h % 2) + 32)
                qr, kr = qrot[h // 2], krot[h // 2]
                st = {"po": None, "sg": {}, "pt": {}}

                def emit_sg(G):
                    gn = min(4, njb - 4 * G)
                    sg = sgp.tile([128, 4, 256], f32, tag="sg")
                    for rj in range(0, gn, 2):      # per 2KB psum region
                        jA, jB = 4 * G + rj, 4 * G + rj + 1
                        mms = [
                            (sg[:, rj + s, 0:W],
                             kr[hp, :, j * 128:j * 128 + 128],
                             qr[hp, :, q0:q0 + W])
                            for s, j in ((0, jA), (1, jB))
                        ]
                        for s, j in ((0, jA), (1, jB)):
                            # diagonal-block causal mask matmul
                            if j >= 2 * c and not (c == 0 and j == 0):
                                which = j - 2 * c   # 0: T0, 1: T1 pattern
                                mms.append(
                                    (sg[:, rj + s, 0:W], ni_s[:],
                                     tm_s[:, :, which, CH - W:CH]))
                        with tc.high_priority():
                            for i, (o, l, r) in enumerate(mms):
                                nc.tensor.matmul(
                                    o, lhsT=l, rhs=r,
                                    start=(i == 0), stop=(i == len(mms) - 1),
                                    perf_mode=DR, skip_group_check=True,
                                )
                    st["sg"][G] = sg

                def emit_expav(G):
                    gn = min(4, njb - 4 * G)
                    pt = ptpool.tile([128, 4, 256], f8, tag="pt")
                    nc.scalar.activation(
                        out=pt[:, 0:gn, 0:W], in_=st["sg"].pop(G)[:, 0:gn, 0:W],
                        func=Exp, scale=EXP_SCALE,
                    )
                    if st["po"] is None:
                        st["po"] = pop.tile([65, 512], f32, tag="po", name="po")
                    po = st["po"]
                    for u in range(0, gn, 2):
                        jb = 4 * G + u
                        nc.tensor.matmul(
                            po[:, 0:W],
                            lhsT=vt[:, jb // 2, h, :, 0:65],
                            rhs=pt[:, u:u + 2, 0:W],
                            start=(jb == 0), stop=(jb + 2 >= njb),
                            perf_mode=DR, skip_group_check=True,
                        )
                    if G == ngrp - 1:
                        # epilogue: normalize by the 16.0-ones denominator
                        # row.  High priority: the po PSUM pool recycles
                        # through this chain, so a lagging epilogue stalls
                        # the AV accumulation two units later.
                        with tc.high_priority():
                            li = lpool.tile([1, 256], f32, tag="li")
                            nc.vector.reciprocal(out=li[:, 0:W], in_=po[64:65, 0:W])
                            lb = lpool.tile([64, 256], f32, tag="lb")
                            nc.gpsimd.partition_broadcast(lb[:, 0:W], li[:, 0:W])
                            hb = 64 * (h % 2)
                            nc.vector.tensor_mul(
                                outTn[h // 2][hb:hb + 64, q0:q0 + W], po[0:64, 0:W],
                                lb[:, 0:W],
                            )

                return ngrp, emit_sg, emit_expav

            def emit_oproj(c, ib):
                ys = yst.tile([128, D_MODEL], f32, tag="ys")
                for ns in range(2):
                    yp = shp.tile([128, 512], f32, tag="pp", name="yp")
                    for pr in range(2):
                        nc.tensor.matmul(
                            yp[:],
                            lhsT=outTn[pr][:, ib * 128:ib * 128 + 128],
                            rhs=wo_s[:, pr, ns * 512:ns * 512 + 512],
                            start=(pr == 0), stop=(pr == 1),
                        )
                    nc.vector.tensor_copy(
                        out=ys[:, ns * 512:ns * 512 + 512], in_=yp[:])
                nc.sync.dma_start(
                    out=y[ib * 128:ib * 128 + 128, :], in_=ys[:])

            # Aux PE work is interleaved at unit boundaries.  Projections are
            # front-loaded (the Activation engine idles until later chunks'
            # scores exist, so finishing all projections early flattens the
            # causal-triangular exp schedule); o_proj items fill afterwards.
            from collections import deque

            projq = deque()                         # (proj_idx, closure)
            for p in (1, 2, 3):
                if p >= 2:
                    projq.append((p, lambda p=p: emit_loads(p)))
                projq.append((p, lambda p=p: emit_proj_kt(p, 0)))
                projq.append((p, lambda p=p: emit_proj_kt(p, 1)))
                projq.append((p, lambda p=p: emit_proj_v(p, 0)))
                projq.append((p, lambda p=p: emit_proj_v(p, 1)))
                if p == 1:
                    for fn in (emit_hi_loads, lambda: emit_hi_qk(0),
                               lambda: emit_hi_qk(1), emit_hi_v, emit_hi_s,
                               emit_hi_av, emit_wo_load):
                        projq.append((1, fn))
            oprojq = deque()                        # ready o_proj items

            def drain_proj(upto):
                while projq and projq[0][0] <= upto:
                    projq.popleft()[1]()

            pending = deque()                       # (emit_expav, G)
            for c in range(nch):
                drain_proj(c // 2)                  # hard dependency
                for h in range(HEADS_PER_CORE):
                    ngrp, emit_sg, emit_expav = make_unit(c, h)
                    for G in range(ngrp):
                        emit_sg(G)
                        pending.append((emit_expav, G))
                        while len(pending) > 1:
                            f, g = pending.popleft()
                            f(g)
                        # boundary aux: prefer projections, two per slot.
                        # none during chunk 0 — early aux wedges the in-order
                        # PE queue behind not-yet-loaded x chunks.
                        for _ in range(2):
                            if projq:
                                projq.popleft()[1]()
                            elif oprojq:
                                oprojq.popleft()()
                if c < nch - 1:
                    oprojq.append(lambda c=c: emit_oproj(c, 2 * c))
                    oprojq.append(lambda c=c: emit_oproj(c, 2 * c + 1))
            while pending:
                f, g = pending.popleft()
                f(g)
            while oprojq:
                oprojq.popleft()()
            emit_oproj(nch - 1, 2 * nch - 2)
            emit_oproj(nch - 1, 2 * nch - 1)

    nc.finalize()
    return nc


def _rope_tables(seq, width, swapped_sign_rows):
    """cos/sin tables in [128, seq] row layout; freq index = row mod 32.
    swapped_sign_rows: '64' -> rows 0-63 get -sin (64-swap layout);
    '32' -> rows [32:64] and [96:128] get -sin (32-swap layout)."""
    half = HEAD_DIM // 2
    inv = 1.0 / (THETA ** (2.0 * np.arange(half) / HEAD_DIM))
    ang = np.arange(seq, dtype=np.float64)[:, None] * inv[None, :]  # [seq, 32]
    cos32 = np.cos(ang).T  # [32, seq]
    sin32 = np.sin(ang).T
    cosI = np.tile(cos32, (4, 1))
    if swapped_sign_rows == "64":
        sinI = np.concatenate([-np.tile(sin32, (2, 1)), np.tile(sin32, (2, 1))], 0)
    else:
        # 32-swap layout: swp rows [od, ev, od', ev'] -> signs [-,+,-,+]
        sinI = np.concatenate([-sin32, sin32, -sin32, sin32], 0)
    return cosI[:, :width], sinI[:, :width]


def make_in_maps(in_features, q_proj, k_proj, v_proj, o_proj, token_positions,
                 seq=SEQ):
    x = np.asarray(in_features, np.float32)
    wq = np.asarray(q_proj, np.float32)
    wk = np.asarray(k_proj, np.float32)
    wv = np.asarray(v_proj, np.float32)
    wo = np.asarray(o_proj, np.float32)

    # fp8-path q/k row perm per 128-row ktile: [h0ev(32) h1ev(32) h0od(32) h1od(32)]
    ev = np.arange(0, HEAD_DIM, 2)
    od = np.arange(1, HEAD_DIM, 2)
    perm8 = []
    for kt in range(2):
        h0, h1 = 2 * kt, 2 * kt + 1
        perm8 += [h0 * HEAD_DIM + ev, h1 * HEAD_DIM + ev,
                  h0 * HEAD_DIM + od, h1 * HEAD_DIM + od]
    perm8 = np.concatenate(perm8)  # local perm within a core's 256 rows
    # hi-path perm: [h0ev h0od h1ev h1od]
    permh = []
    for kt in range(2):
        h0, h1 = 2 * kt, 2 * kt + 1
        permh += [h0 * HEAD_DIM + ev, h0 * HEAD_DIM + od,
                  h1 * HEAD_DIM + ev, h1 * HEAD_DIM + od]
    permh = np.concatenate(permh)

    npc = seq // PC
    # x chunked fp8: [npc, 128, 8, PC]
    xt8b, xhib = [], []
    for b in range(x.shape[0]):
        xT = np.ascontiguousarray(x[b].T)                       # [1024, seq]
        xt8b.append(np.ascontiguousarray(
            xT.reshape(8, 128, npc, PC).transpose(2, 1, 0, 3)).astype(F8))
        xhib.append(np.ascontiguousarray(
            xT[:, 0:128].reshape(8, 128, 128).transpose(1, 0, 2)).astype(BF16))

    # rope tables, fp8 path (64-swap): [npc, 128, 2, PC]
    cosI, sinI = _rope_tables(seq, seq, "64")
    tabf = np.stack([cosI, sinI], axis=1)                       # [128, 2, seq]
    tabf = np.ascontiguousarray(
        tabf.reshape(128, 2, npc, PC).transpose(2, 0, 1, 3)).astype(BF16)
    # hi tables (32-swap), width 128
    cosH, sinH = _rope_tables(seq, 128, "32")
    htabf = np.ascontiguousarray(np.stack([cosH, sinH], 1)).astype(BF16)

    # causal mask patterns T0/T1 [128, CH], value MASKV
    k_ = np.arange(128)[:, None]
    j_ = np.arange(CH)[None, :]
    T0 = (j_ < k_).astype(np.float32) * MASKV
    T1 = (j_ < k_ + 128).astype(np.float32) * MASKV
    tmskf = np.ascontiguousarray(
        np.broadcast_to(np.stack([T0, T1], 0)[None], (2, 2, 128, CH))
        .transpose(2, 0, 1, 3)).astype(F8)
    negif = np.ascontiguousarray(
        np.broadcast_to((-MASKV * np.eye(128, dtype=np.float32))[:, None, :],
                        (128, 2, 128))).astype(F8)

    def wtile(w, perm, scale, dtype):
        # rows ks (already core-sliced, perm applied), transposed -> [128, 8, 256]
        wT = np.ascontiguousarray(w[perm].T)                    # [1024, 256]
        return np.ascontiguousarray(
            (wT * scale).reshape(8, 128, KSLICE).transpose(1, 0, 2)).astype(dtype)

    in_maps = []
    for core in range(N_CORES):
        b, g = divmod(core, HEADS_PER_CORE)
        ks = np.arange(g * KSLICE, (g + 1) * KSLICE)
        wq_c, wk_c, wv_c = wq[ks], wk[ks], wv[ks]
        in_maps.append({
            "xt8": xt8b[b],
            "wq8": wtile(wq_c, perm8, WSCALE, F8),
            "wk8": wtile(wk_c, perm8, WSCALE, F8),
            "wv8": wtile(wv_c, np.arange(KSLICE), WSCALE, F8),
            "tab": tabf,
            "tmsk": tmskf,
            "negi": negif,
            "woT": np.ascontiguousarray(
                wo[:, ks].T.reshape(2, 128, D_MODEL)).astype(np.float32),
            "xhi": xhib[b],
            "whq": wtile(wq_c, permh, 1.0, BF16),
            "whk": wtile(wk_c, permh, 1.0, BF16),
            "whv": wtile(wv_c, np.arange(KSLICE), 1.0, BF16),
            "htab": htabf,
        })
    return in_maps


def assemble(results, batch=2):
    ys = []
    for b in range(batch):
        parts = [results[b * HEADS_PER_CORE + g]["y"].astype(np.float64)
                 for g in range(HEADS_PER_CORE)]
        ys.append(np.sum(parts, axis=0, dtype=np.float64).astype(np.float32))
    return np.stack(ys, axis=0)


_NC_CACHE = {}


def get_nc(seq=SEQ):
    import os
    key = (seq, os.environ.get("KBISECT", "3"))
    if key not in _NC_CACHE:
        _NC_CACHE[key] = build_nc(seq)
    return _NC_CACHE[key]


def kernel(**inputs):
    from concourse.bass_utils import run_bass_kernel_spmd

    nc = get_nc()
    in_maps = make_in_maps(**inputs)
    res = run_bass_kernel_spmd(nc, in_maps, list(range(N_CORES)))
    return assemble(res.results)


if __name__ == "__main__":
    rng = np.random.default_rng(0)
    ins = {
        "in_features": rng.standard_normal((2, SEQ, D_MODEL), np.float32),
        "q_proj": (rng.standard_normal((D_MODEL, D_MODEL)) * 0.02).astype(np.float32),
        "k_proj": (rng.standard_normal((D_MODEL, D_MODEL)) * 0.02).astype(np.float32),
        "v_proj": (rng.standard_normal((D_MODEL, D_MODEL)) * 0.02).astype(np.float32),
        "o_proj": (rng.standard_normal((D_MODEL, D_MODEL)) * 0.02).astype(np.float32),
        "token_positions": np.arange(SEQ, dtype=np.int32),
    }
    out = kernel(**ins)
    print("kernel output:", out.shape, out.dtype)


# revision 68
# speedup vs baseline: 1.0629x; 1.0060x over previous
"""Causal multi-head attention (RoPE) on 8 Trainium2 NeuronCores — fp8 edition.

Sharding: (batch=2) x (head groups=4) -> 8 cores; core c = 4*b + g handles
batch b, heads [4g, 4g+4). Each core computes its 4 heads' attention plus its
partial o_proj contribution; the host sums 4 partials per batch.

Per-core kernel (single Tile program, SPMD over cores):
  - QKV projections and the S / AV attention matmuls run in fp8e4m3 with
    MatmulPerfMode.DoubleRow (2 contraction tiles per instruction at 0.5
    cycles/row -> 4x the f32r/bf16 matmul throughput).  Weights are
    pre-scaled by 16 on the host to center them in fp8 range; the scale is
    folded into the exp() activation scale (2^-11) and a 16.0 ones-column
    that yields the softmax denominator.
  - Queries/keys 0-127 run through a separate bf16 "hi-precision" path
    (fp8 noise does not average out over few softmax terms at early rows).
  - Causal masking of diagonal key-blocks is done on the PE: a constant
    fp8 DoubleRow matmul accumulates -115200 (* 2^-11 -> -56) into masked
    score positions, so exp() flushes them to exact zero.  No vector-engine
    mask pass.
  - o_proj stays f32r (accuracy); softmax epilogue = reciprocal (DVE) +
    partition_broadcast (Pool) + multiply (DVE).
  - RoPE: PSUM->SBUF bf16 copy (DVE), 64-partition swap via SBUF-SBUF DMA,
    bf16 table multiplies (DVE 2x mode), two adds -> fp8 (Pool).
  - Emission is software-pipelined: the 32 (chunk, head) attention units
    form one stream with one score-group of lookahead, and projection /
    hi-path / o_proj work is spread across unit boundaries (projections
    front-loaded) so the Activation engine — the critical engine at ~78 us
    of exp() — stays fed.  Hardware quirks found on the way: GPSIMD cannot
    touch PSUM; a PSUM accumulation bracket must keep one tile_position
    (mixing base partitions crashes the device); DoubleRow Ldweights needs
    a >=128-element pair stride.

Layouts (per core):
  qrot/krot: two [64, 2, seq] fp8 tiles (head pair kt); head h on
    partitions 32(h%2)..+32, plane t in {0,1} = rotated-even/odd dims;
    S = DoubleRow over (32 partitions x 2 planes) = the 64-dim contraction.
  vt [128, 8, 4, 2, 128] fp8: key-block pair, head, block-in-pair; cols
    0:64 = v-dims, col 64 = 16.0 ones column (softmax denominator).
  S^T tiles [128 keys, W queries] accumulate 4 key-blocks per PSUM group;
  one exp() per group feeds DoubleRow AV over key-block pairs.
"""

import sys

for _p in ("/opt/trn_rl_repo",):
    if _p not in sys.path:
        sys.path.insert(0, _p)

import numpy as np
import ml_dtypes

F8 = ml_dtypes.float8_e4m3
BF16 = ml_dtypes.bfloat16

SEQ = 2048
D_MODEL = 1024
NUM_HEADS = 16
HEAD_DIM = 64
THETA = 10000.0
N_CORES = 8
HEADS_PER_CORE = 4
KSLICE = HEADS_PER_CORE * HEAD_DIM  # 256 projection rows per core
PC = 512          # projection chunk width (4 chunks)
CH = 256          # attention query-chunk width (8 chunks)
WSCALE = 16.0     # host pre-scale on wq/wk/wv (fp8 range centering)
EXP_SCALE = 0.125 / (WSCALE * WSCALE)  # folds q*k scale^2 and 1/sqrt(hd)
MASKV = 240.0     # fp8e4m3 max-ish; DR mask adds -2*240*240 = -115200


def build_nc(seq=SEQ):
    import os
    BISECT = int(os.environ.get("KBISECT", "3"))  # 1: proj only, 2: +S/exp, 3: full
    KHI = int(os.environ.get("KHI", "1"))
    KHIPART = int(os.environ.get("KHIPART", "4"))
    KPROJ = int(os.environ.get("KPROJ", "1"))
    KOPROJ = int(os.environ.get("KOPROJ", "1"))
    import concourse.mybir as mybir
    import concourse.tile as tile
    from concourse import bacc
    from contextlib import ExitStack

    f32 = mybir.dt.float32
    f32r = mybir.dt.float32r
    bf16 = mybir.dt.bfloat16
    f8 = mybir.dt.float8e4
    DR = mybir.MatmulPerfMode.DoubleRow
    Exp = mybir.ActivationFunctionType.Exp

    npc = seq // PC              # 4 projection chunks
    nch = seq // CH              # 8 attention chunks
    nib = seq // 128             # 16 key/row blocks

    nc = bacc.Bacc(None, target_bir_lowering=False)

    # fp8 path inputs
    xt8 = nc.declare_dram_parameter("xt8", [npc, 128, 8, PC], f8, isOutput=False)
    wq8 = nc.declare_dram_parameter("wq8", [128, 8, KSLICE], f8, isOutput=False)
    wk8 = nc.declare_dram_parameter("wk8", [128, 8, KSLICE], f8, isOutput=False)
    wv8 = nc.declare_dram_parameter("wv8", [128, 8, KSLICE], f8, isOutput=False)
    tab = nc.declare_dram_parameter("tab", [npc, 128, 2, PC], bf16, isOutput=False)
    tmsk = nc.declare_dram_parameter("tmsk", [128, 2, 2, CH], f8, isOutput=False)
    negi = nc.declare_dram_parameter("negi", [128, 2, 128], f8, isOutput=False)
    # o_proj (f32r)
    woT = nc.declare_dram_parameter("woT", [2, 128, D_MODEL], f32r, isOutput=False)
    # hi-precision path (rows/keys 0-127), bf16
    xhi = nc.declare_dram_parameter("xhi", [128, 8, 128], bf16, isOutput=False)
    whq = nc.declare_dram_parameter("whq", [128, 8, KSLICE], bf16, isOutput=False)
    whk = nc.declare_dram_parameter("whk", [128, 8, KSLICE], bf16, isOutput=False)
    whv = nc.declare_dram_parameter("whv", [128, 8, KSLICE], bf16, isOutput=False)
    htab = nc.declare_dram_parameter("htab", [128, 2, 128], bf16, isOutput=False)

    y = nc.declare_dram_parameter("y", [seq, D_MODEL], f32, isOutput=True)

    with tile.TileContext(nc) as tc, ExitStack() as ctx:
        persist = ctx.enter_context(tc.tile_pool(name="persist", bufs=1))

        qrot = [persist.tile([64, 2, seq], f8, tag=f"qrot{k}", name=f"qrot{k}")
                for k in range(2)]
        krot = [persist.tile([64, 2, seq], f8, tag=f"krot{k}", name=f"krot{k}")
                for k in range(2)]
        # AV DoubleRow weights: k-block PAIR on dim3 with 128-wide inner
        # stride (walrus ISA check rejects <128 pair strides); cols 0:64 =
        # v-dims, col 64 = 16.0 ones (softmax denominator), 65:128 unused.
        vt = persist.tile([128, nib // 2, HEADS_PER_CORE, 2, 128], f8, tag="vt")
        outTn = [persist.tile([128, seq], f32r, tag=f"outTnP{p}", name=f"outTnP{p}")
                 for p in range(HEADS_PER_CORE // 2)]
        nc.vector.memset(vt[:, :, :, :, 64:65], WSCALE)

        wq_s = persist.tile([128, 8, KSLICE], f8, tag="wq_s")
        wk_s = persist.tile([128, 8, KSLICE], f8, tag="wk_s")
        wv_s = persist.tile([128, 8, KSLICE], f8, tag="wv_s")
        wo_s = persist.tile([128, 2, D_MODEL], f32r, tag="wo_s")
        tm_s = persist.tile([128, 2, 2, CH], f8, tag="tm_s")
        ni_s = persist.tile([128, 2, 128], f8, tag="ni_s")
        # load order = dependency order: first projection chunk's x and the
        # q/k weights gate everything; wo/o_proj and hi-path gear come later.
        nc.sync.dma_start(out=wq_s[:], in_=wq8[:])
        nc.sync.dma_start(out=wk_s[:], in_=wk8[:])

        # hi-path persistent
        xhi_s = persist.tile([128, 8, 128], bf16, tag="xhi_s")
        whq_s = persist.tile([128, 8, KSLICE], bf16, tag="whq_s")
        whk_s = persist.tile([128, 8, KSLICE], bf16, tag="whk_s")
        whv_s = persist.tile([128, 8, KSLICE], bf16, tag="whv_s")
        htab_s = persist.tile([128, 2, 128], bf16, tag="htab_s")
        qhi = [persist.tile([128, 128], bf16, tag=f"qhi{k}", name=f"qhi{k}") for k in range(2)]
        khi = [persist.tile([128, 128], bf16, tag=f"khi{k}", name=f"khi{k}") for k in range(2)]
        vthi = persist.tile([128, HEADS_PER_CORE, 65], bf16, tag="vthi")
        nc.vector.memset(vthi[:, :, 64:65], 1.0)

        with (
            tc.tile_pool(name="p1x", bufs=2) as p1x,
            tc.tile_pool(name="rtmp", bufs=8) as rtmp,
            tc.tile_pool(name="ptpool", bufs=4) as ptpool,
            tc.tile_pool(name="lpool", bufs=6) as lpool,
            tc.tile_pool(name="ystage", bufs=4) as yst,
            tc.tile_pool(name="sgpsum", bufs=2, space="PSUM") as sgp,
            tc.tile_pool(name="popsum", bufs=2, space="PSUM") as pop,
            tc.tile_pool(name="shpsum", bufs=2, space="PSUM") as shp,
        ):
            if BISECT < 3 or not KHI:
                zt = rtmp.tile([128, 512], f32, tag="zfill")
                nc.vector.memset(zt[:], 0.0)
                for p_ in range(2):
                    for cc in range(4):
                        nc.vector.tensor_copy(
                            out=outTn[p_][:, cc * 512:cc * 512 + 512], in_=zt[:])

            # ---------------- hi-precision path: rows/keys 0-127 ------------
            # Emitted as aux items interleaved with early attention chunks
            # (its long serial chain would otherwise stall the pipe start).
            U = persist.tile([128, 128], f32, tag="umask")
            nc.gpsimd.memset(U[:], 0.0)
            nc.gpsimd.affine_select(
                out=U[:], in_=U[:], compare_op=mybir.AluOpType.is_ge,
                fill=-1e9, base=0, pattern=[[1, 128]], channel_multiplier=-1,
            )
            pthi = ptpool.tile([128, 4, 128], bf16, tag="pthi")

            def emit_hi_qk(kt):
                ctxh = tc.high_priority(offset=500)
                ctxh.__enter__()
                # projections (bf16, contraction d=1024 over 8 d-tiles)
                for w_s, dst in ((whq_s, qhi[kt]), (whk_s, khi[kt])):
                    pp = shp.tile([128, 512], f32, tag="pp", name=f"hiqk{kt}")
                    for dt in range(8):
                        nc.tensor.matmul(
                            pp[:, 0:128],
                            lhsT=w_s[:, dt, kt * 128:kt * 128 + 128],
                            rhs=xhi_s[:, dt, :],
                            start=(dt == 0), stop=(dt == 7),
                        )
                    # rope (baseline-perm rows: [h0ev h0od h1ev h1od], 32-swap)
                    praw = rtmp.tile([128, 128], bf16, tag="hpraw")
                    nc.vector.tensor_copy(out=praw[:], in_=pp[:, 0:128])
                    swp = rtmp.tile([128, 128], bf16, tag="hswp")
                    for blk in range(4):
                        d, s = blk * 32, blk * 32 + (32 if blk % 2 == 0 else -32)
                        nc.vector.tensor_copy(out=swp[d:d + 32, :], in_=praw[s:s + 32, :])
                    pc_ = rtmp.tile([128, 128], bf16, tag="hpc")
                    ps_ = rtmp.tile([128, 128], bf16, tag="hps")
                    nc.gpsimd.tensor_mul(pc_[:], praw[:], htab_s[:, 0, :])
                    nc.gpsimd.tensor_mul(ps_[:], swp[:], htab_s[:, 1, :])
                    nc.vector.tensor_add(dst[:], pc_[:], ps_[:])
                ctxh.__exit__(None, None, None)

            def emit_hi_v():
                vp = shp.tile([128, KSLICE], f32, tag="pp", name="hiv")
                for dt in range(8):
                    nc.tensor.matmul(
                        vp[:], lhsT=xhi_s[:, dt, :], rhs=whv_s[:, dt, :],
                        start=(dt == 0), stop=(dt == 7),
                    )
                nc.scalar.copy(
                    out=vthi[:, :, 0:64],
                    in_=vp[:].rearrange("p (h z) -> p h z", z=64),
                )

            def emit_hi_s():
                # NB: one accumulation bracket per head — mixing tile_position
                # (base partition) inside a PSUM bracket crashes the hardware.
                for h in range(4):
                    shi = shp.tile([128, 512], f32, tag="pp", name=f"shi{h}")
                    nc.tensor.matmul(
                        shi[:, 0:128],
                        lhsT=khi[h // 2][64 * (h % 2):64 * (h % 2) + 64, :],
                        rhs=qhi[h // 2][64 * (h % 2):64 * (h % 2) + 64, :],
                        start=True, stop=True,
                    )
                    nc.vector.tensor_add(shi[:, 0:128], shi[:, 0:128], U[:])
                    nc.scalar.activation(out=pthi[:, h, :], in_=shi[:, 0:128],
                                         func=Exp, scale=0.125)

            def emit_hi_av():
                pohi = shp.tile([65, 4, 128], f32, tag="pp", name="pohi")
                for h in range(4):
                    nc.tensor.matmul(
                        pohi[:, h, :], lhsT=vthi[:, h, :], rhs=pthi[:, h, :],
                        start=(h == 0), stop=(h == 3),
                    )
                for h in range(4):
                    li = lpool.tile([1, 128], f32, tag="li", name="hli")
                    nc.vector.reciprocal(out=li[:], in_=pohi[64:65, h, :])
                    lb = lpool.tile([64, 128], f32, tag="lb", name="hlb")
                    nc.gpsimd.partition_broadcast(lb[:], li[:])
                    hb = 64 * (h % 2)
                    nc.vector.tensor_mul(
                        outTn[h // 2][hb:hb + 64, 0:128], pohi[0:64, h, :], lb[:]
                    )

            # ---------------- fp8 projections, per 512-chunk ----------------
            xts, tbs = {}, {}

            def emit_loads(p):
                if p >= npc or p in xts:
                    return
                xt = p1x.tile([128, 8, PC], f8, tag="xt")
                nc.sync.dma_start(out=xt[:], in_=xt8[p])
                tb = p1x.tile([128, 2, PC], bf16, tag="tb")
                nc.sync.dma_start(out=tb[:], in_=tab[p])
                xts[p], tbs[p] = xt, tb

            def emit_proj_kt(p, kt):
                """Q and K projection + rope for head-pair kt of chunk p,
                phase-interleaved (both PSUM copies run while the swap DMAs
                fly, so the table-multiplies rarely stall).  High priority:
                this chain gates whole chunks of attention."""
                xt, tb = xts[p], tbs[p]
                csl = slice(p * PC, p * PC + PC)
                pps, praws, swps = [], [], []
                for w_s in (wq_s, wk_s):
                    # one PSUM group across both 256-wide halves (shared 2KB
                    # zero region: start once, stop on the final matmul)
                    pp = shp.tile([128, 512], f32, tag="pp")
                    for hf in range(2):
                        for a in range(4):
                            nc.tensor.matmul(
                                pp[:, hf * 256:hf * 256 + 256],
                                lhsT=w_s[:, 2 * a:2 * a + 2, kt * 128:kt * 128 + 128],
                                rhs=xt[:, 2 * a:2 * a + 2, hf * 256:hf * 256 + 256],
                                start=(hf == 0 and a == 0),
                                stop=(hf == 1 and a == 3),
                                perf_mode=DR, skip_group_check=True,
                            )
                    pps.append(pp)
                for i in range(2):
                    praw = rtmp.tile([128, PC], bf16, tag="praw")
                    nc.vector.tensor_copy(out=praw[:], in_=pps[i][:])
                    praws.append(praw)
                    swp = rtmp.tile([128, PC], bf16, tag="swp")
                    nc.sync.dma_start(out=swp[0:64, :], in_=praw[64:128, :])
                    nc.sync.dma_start(out=swp[64:128, :], in_=praw[0:64, :])
                    swps.append(swp)
                for i, rot in enumerate((qrot, krot)):
                    pc_ = rtmp.tile([128, PC], bf16, tag="pc")
                    ps_ = rtmp.tile([128, PC], bf16, tag="ps")
                    nc.vector.tensor_mul(pc_[:], praws[i][:], tb[:, 0, :])
                    nc.vector.tensor_mul(ps_[:], swps[i][:], tb[:, 1, :])
                    # heads (2kt, 2kt+1) live in tile kt on partitions 0-63;
                    # plane t0=even-rot, t1=odd-rot.
                    # praw rows: [h0ev h1ev | h0od h1od].
                    nc.gpsimd.tensor_add(
                        rot[kt][:, 0, csl], pc_[0:64, :], ps_[0:64, :])
                    nc.gpsimd.tensor_add(
                        rot[kt][:, 1, csl], pc_[64:128, :], ps_[64:128, :])

            def emit_proj_v(p, half):
                """V projection for 2 of chunk p's 4 position-blocks."""
                xt = xts[p]
                for ibl in (2 * half, 2 * half + 1):
                    ib = p * (PC // 128) + ibl
                    vp = shp.tile([128, KSLICE], f32, tag="pp", name="vp")
                    for a in range(4):
                        nc.tensor.matmul(
                            vp[:],
                            lhsT=xt[:, 2 * a:2 * a + 2, ibl * 128:ibl * 128 + 128],
                            rhs=wv_s[:, 2 * a:2 * a + 2, :],
                            start=(a == 0), stop=(a == 3),
                            perf_mode=DR, skip_group_check=True,
                        )
                    nc.scalar.copy(
                        out=vt[:, ib // 2, :, ib % 2, 0:64],
                        in_=vp[:].rearrange("p (h z) -> p h z", z=64),
                    )

            def emit_proj(p):
                emit_proj_kt(p, 0)
                emit_proj_kt(p, 1)
                emit_proj_v(p, 0)
                emit_proj_v(p, 1)

            emit_loads(0)
            emit_loads(1)
            nc.sync.dma_start(out=wv_s[:], in_=wv8[:])
            nc.sync.dma_start(out=tm_s[:], in_=tmsk[:])
            nc.sync.dma_start(out=ni_s[:], in_=negi[:])
            emit_proj(0)

            def emit_hi_loads():
                nc.sync.dma_start(out=xhi_s[:], in_=xhi[:])
                nc.sync.dma_start(out=whq_s[:], in_=whq[:])
                nc.sync.dma_start(out=whk_s[:], in_=whk[:])
                nc.sync.dma_start(out=whv_s[:], in_=whv[:])
                nc.sync.dma_start(out=htab_s[:], in_=htab[:])

            def emit_wo_load():
                nc.sync.dma_start(out=wo_s[:], in_=woT[:].rearrange("q p d -> p q d"))

            # ---------------- attention: pipelined (chunk, head) units -------
            def make_unit(c, h):
                """Returns (ngrp, emit_sg, emit_expav) closures for one
                attention unit: chunk c (q-window), head h."""
                q0 = c * CH + (128 if c == 0 else 0)   # hi-path covers rows 0-127
                W = c * CH + CH - q0
                njb = 2 * (c + 1)                       # causal key blocks
                ngrp = (njb + 3) // 4
                hp = slice(32 * (h % 2), 32 * (h % 2) + 32)
                qr, kr = qrot[h // 2], krot[h // 2]
                st = {"po": None, "sg": {}, "pt": {}}

                def emit_sg(G):
                    gn = min(4, njb - 4 * G)
                    sg = sgp.tile([128, 4, 256], f32, tag="sg")
                    for rj in range(0, gn, 2):      # per 2KB psum region
                        jA, jB = 4 * G + rj, 4 * G + rj + 1
                        mms = [
                            (sg[:, rj + s, 0:W],
                             kr[hp, :, j * 128:j * 128 + 128],
                             qr[hp, :, q0:q0 + W])
                            for s, j in ((0, jA), (1, jB))
                        ]
                        for s, j in ((0, jA), (1, jB)):
                            # diagonal-block causal mask matmul
                            if j >= 2 * c and not (c == 0 and j == 0):
                                which = j - 2 * c   # 0: T0, 1: T1 pattern
                                mms.append(
                                    (sg[:, rj + s, 0:W], ni_s[:],
                                     tm_s[:, :, which, CH - W:CH]))
                        with tc.high_priority():
                            for i, (o, l, r) in enumerate(mms):
                                nc.tensor.matmul(
                                    o, lhsT=l, rhs=r,
                                    start=(i == 0), stop=(i == len(mms) - 1),
                                    perf_mode=DR, skip_group_check=True,
                                )
                    st["sg"][G] = sg

                def emit_expav(G):
                    gn = min(4, njb - 4 * G)
                    pt = ptpool.tile([128, 4, 256], f8, tag="pt")
                    nc.scalar.activation(
                        out=pt[:, 0:gn, 0:W], in_=st["sg"].pop(G)[:, 0:gn, 0:W],
                        func=Exp, scale=EXP_SCALE,
                    )
                    if st["po"] is None:
                        st["po"] = pop.tile([65, 512], f32, tag="po", name="po")
                    po = st["po"]
                    for u in range(0, gn, 2):
                        jb = 4 * G + u
                        nc.tensor.matmul(
                            po[:, 0:W],
                            lhsT=vt[:, jb // 2, h, :, 0:65],
                            rhs=pt[:, u:u + 2, 0:W],
                            start=(jb == 0), stop=(jb + 2 >= njb),
                            perf_mode=DR, skip_group_check=True,
                        )
                    if G == ngrp - 1:
                        # epilogue: normalize by the 16.0-ones denominator
                        # row.  High priority: the po PSUM pool recycles
                        # through this chain, so a lagging epilogue stalls
                        # the AV accumulation two units later.
                        with tc.high_priority():
                            li = lpool.tile([1, 256], f32, tag="li")
                            nc.vector.reciprocal(out=li[:, 0:W], in_=po[64:65, 0:W])
                            lb = lpool.tile([64, 256], f32, tag="lb")
                            nc.gpsimd.partition_broadcast(lb[:, 0:W], li[:, 0:W])
                            hb = 64 * (h % 2)
                            nc.vector.tensor_mul(
                                outTn[h // 2][hb:hb + 64, q0:q0 + W], po[0:64, 0:W],
                                lb[:, 0:W],
                            )

                return ngrp, emit_sg, emit_expav

            def emit_oproj(c, ib):
                ys = yst.tile([128, D_MODEL], f32, tag="ys")
                for ns in range(2):
                    yp = shp.tile([128, 512], f32, tag="pp", name="yp")
                    for pr in range(2):
                        nc.tensor.matmul(
                            yp[:],
                            lhsT=outTn[pr][:, ib * 128:ib * 128 + 128],
                            rhs=wo_s[:, pr, ns * 512:ns * 512 + 512],
                            start=(pr == 0), stop=(pr == 1),
                        )
                    nc.vector.tensor_copy(
                        out=ys[:, ns * 512:ns * 512 + 512], in_=yp[:])
                nc.sync.dma_start(
                    out=y[ib * 128:ib * 128 + 128, :], in_=ys[:])

            # Aux PE work is interleaved at unit boundaries.  Projections are
            # front-loaded (the Activation engine idles until later chunks'
            # scores exist, so finishing all projections early flattens the
            # causal-triangular exp schedule); o_proj items fill afterwards.
            from collections import deque

            projq = deque()                         # (proj_idx, closure)
            for p in (1, 2, 3):
                if p >= 2:
                    projq.append((p, lambda p=p: emit_loads(p)))
                projq.append((p, lambda p=p: emit_proj_kt(p, 0)))
                projq.append((p, lambda p=p: emit_proj_kt(p, 1)))
                projq.append((p, lambda p=p: emit_proj_v(p, 0)))
                projq.append((p, lambda p=p: emit_proj_v(p, 1)))
                if p == 1:
                    for fn in (emit_hi_loads, lambda: emit_hi_qk(0),
                               lambda: emit_hi_qk(1), emit_hi_v, emit_hi_s,
                               emit_hi_av, emit_wo_load):
                        projq.append((1, fn))
            oprojq = deque()                        # ready o_proj items

            def drain_proj(upto):
                while projq and projq[0][0] <= upto:
                    projq.popleft()[1]()

            pending = deque()                       # (emit_expav, G)
            for c in range(nch):
                drain_proj(c // 2)                  # hard dependency
                for h in range(HEADS_PER_CORE):
                    ngrp, emit_sg, emit_expav = make_unit(c, h)
                    for G in range(ngrp):
                        emit_sg(G)
                        pending.append((emit_expav, G))
                        while len(pending) > 0:
                            f, g = pending.popleft()
                            f(g)
                        # boundary aux: prefer projections, two per slot.
                        # none during chunk 0 — early aux wedges the in-order
                        # PE queue behind not-yet-loaded x chunks.
                        for _ in range(2):
                            if projq:
                                projq.popleft()[1]()
                            elif oprojq:
                                oprojq.popleft()()
                if c < nch - 1:
                    oprojq.append(lambda c=c: emit_oproj(c, 2 * c))
                    oprojq.append(lambda c=c: emit_oproj(c, 2 * c + 1))
            while pending:
                f, g = pending.popleft()
                f(g)
            while oprojq:
                oprojq.popleft()()
            emit_oproj(nch - 1, 2 * nch - 2)
            emit_oproj(nch - 1, 2 * nch - 1)

    nc.finalize()
    return nc


def _rope_tables(seq, width, swapped_sign_rows):
    """cos/sin tables in [128, seq] row layout; freq index = row mod 32.
    swapped_sign_rows: '64' -> rows 0-63 get -sin (64-swap layout);
    '32' -> rows [32:64] and [96:128] get -sin (32-swap layout)."""
    half = HEAD_DIM // 2
    inv = 1.0 / (THETA ** (2.0 * np.arange(half) / HEAD_DIM))
    ang = np.arange(seq, dtype=np.float64)[:, None] * inv[None, :]  # [seq, 32]
    cos32 = np.cos(ang).T  # [32, seq]
    sin32 = np.sin(ang).T
    cosI = np.tile(cos32, (4, 1))
    if swapped_sign_rows == "64":
        sinI = np.concatenate([-np.tile(sin32, (2, 1)), np.tile(sin32, (2, 1))], 0)
    else:
        # 32-swap layout: swp rows [od, ev, od', ev'] -> signs [-,+,-,+]
        sinI = np.concatenate([-sin32, sin32, -sin32, sin32], 0)
    return cosI[:, :width], sinI[:, :width]


def make_in_maps(in_features, q_proj, k_proj, v_proj, o_proj, token_positions,
                 seq=SEQ):
    x = np.asarray(in_features, np.float32)
    wq = np.asarray(q_proj, np.float32)
    wk = np.asarray(k_proj, np.float32)
    wv = np.asarray(v_proj, np.float32)
    wo = np.asarray(o_proj, np.float32)

    # fp8-path q/k row perm per 128-row ktile: [h0ev(32) h1ev(32) h0od(32) h1od(32)]
    ev = np.arange(0, HEAD_DIM, 2)
    od = np.arange(1, HEAD_DIM, 2)
    perm8 = []
    for kt in range(2):
        h0, h1 = 2 * kt, 2 * kt + 1
        perm8 += [h0 * HEAD_DIM + ev, h1 * HEAD_DIM + ev,
                  h0 * HEAD_DIM + od, h1 * HEAD_DIM + od]
    perm8 = np.concatenate(perm8)  # local perm within a core's 256 rows
    # hi-path perm: [h0ev h0od h1ev h1od]
    permh = []
    for kt in range(2):
        h0, h1 = 2 * kt, 2 * kt + 1
        permh += [h0 * HEAD_DIM + ev, h0 * HEAD_DIM + od,
                  h1 * HEAD_DIM + ev, h1 * HEAD_DIM + od]
    permh = np.concatenate(permh)

    npc = seq // PC
    # x chunked fp8: [npc, 128, 8, PC]
    xt8b, xhib = [], []
    for b in range(x.shape[0]):
        xT = np.ascontiguousarray(x[b].T)                       # [1024, seq]
        xt8b.append(np.ascontiguousarray(
            xT.reshape(8, 128, npc, PC).transpose(2, 1, 0, 3)).astype(F8))
        xhib.append(np.ascontiguousarray(
            xT[:, 0:128].reshape(8, 128, 128).transpose(1, 0, 2)).astype(BF16))

    # rope tables, fp8 path (64-swap): [npc, 128, 2, PC]
    cosI, sinI = _rope_tables(seq, seq, "64")
    tabf = np.stack([cosI, sinI], axis=1)                       # [128, 2, seq]
    tabf = np.ascontiguousarray(
        tabf.reshape(128, 2, npc, PC).transpose(2, 0, 1, 3)).astype(BF16)
    # hi tables (32-swap), width 128
    cosH, sinH = _rope_tables(seq, 128, "32")
    htabf = np.ascontiguousarray(np.stack([cosH, sinH], 1)).astype(BF16)

    # causal mask patterns T0/T1 [128, CH], value MASKV
    k_ = np.arange(128)[:, None]
    j_ = np.arange(CH)[None, :]
    T0 = (j_ < k_).astype(np.float32) * MASKV
    T1 = (j_ < k_ + 128).astype(np.float32) * MASKV
    tmskf = np.ascontiguousarray(
        np.broadcast_to(np.stack([T0, T1], 0)[None], (2, 2, 128, CH))
        .transpose(2, 0, 1, 3)).astype(F8)
    negif = np.ascontiguousarray(
        np.broadcast_to((-MASKV * np.eye(128, dtype=np.float32))[:, None, :],
                        (128, 2, 128))).astype(F8)

    def wtile(w, perm, scale, dtype):
        # rows ks (already core-sliced, perm applied), transposed -> [128, 8, 256]
        wT = np.ascontiguousarray(w[perm].T)                    # [1024, 256]
        return np.ascontiguousarray(
            (wT * scale).reshape(8, 128, KSLICE).transpose(1, 0, 2)).astype(dtype)

    in_maps = []
    for core in range(N_CORES):
        b, g = divmod(core, HEADS_PER_CORE)
        ks = np.arange(g * KSLICE, (g + 1) * KSLICE)
        wq_c, wk_c, wv_c = wq[ks], wk[ks], wv[ks]
        in_maps.append({
            "xt8": xt8b[b],
            "wq8": wtile(wq_c, perm8, WSCALE, F8),
            "wk8": wtile(wk_c, perm8, WSCALE, F8),
            "wv8": wtile(wv_c, np.arange(KSLICE), WSCALE, F8),
            "tab": tabf,
            "tmsk": tmskf,
            "negi": negif,
            "woT": np.ascontiguousarray(
                wo[:, ks].T.reshape(2, 128, D_MODEL)).astype(np.float32),
            "xhi": xhib[b],
            "whq": wtile(wq_c, permh, 1.0, BF16),
            "whk": wtile(wk_c, permh, 1.0, BF16),
            "whv": wtile(wv_c, np.arange(KSLICE), 1.0, BF16),
            "htab": htabf,
        })
    return in_maps


def assemble(results, batch=2):
    ys = []
    for b in range(batch):
        parts = [results[b * HEADS_PER_CORE + g]["y"].astype(np.float64)
                 for g in range(HEADS_PER_CORE)]
        ys.append(np.sum(parts, axis=0, dtype=np.float64).astype(np.float32))
    return np.stack(ys, axis=0)


_NC_CACHE = {}


def get_nc(seq=SEQ):
    import os
    key = (seq, os.environ.get("KBISECT", "3"))
    if key not in _NC_CACHE:
        _NC_CACHE[key] = build_nc(seq)
    return _NC_CACHE[key]


def kernel(**inputs):
    from concourse.bass_utils import run_bass_kernel_spmd

    nc = get_nc()
    in_maps = make_in_maps(**inputs)
    res = run_bass_kernel_spmd(nc, in_maps, list(range(N_CORES)))
    return assemble(res.results)


if __name__ == "__main__":
    rng = np.random.default_rng(0)
    ins = {
        "in_features": rng.standard_normal((2, SEQ, D_MODEL), np.float32),
        "q_proj": (rng.standard_normal((D_MODEL, D_MODEL)) * 0.02).astype(np.float32),
        "k_proj": (rng.standard_normal((D_MODEL, D_MODEL)) * 0.02).astype(np.float32),
        "v_proj": (rng.standard_normal((D_MODEL, D_MODEL)) * 0.02).astype(np.float32),
        "o_proj": (rng.standard_normal((D_MODEL, D_MODEL)) * 0.02).astype(np.float32),
        "token_positions": np.arange(SEQ, dtype=np.int32),
    }
    out = kernel(**ins)
    print("kernel output:", out.shape, out.dtype)
